# revision 1
# baseline (speedup 1.0000x reference)
"""Trainium2 Bass kernel for nn_CMAF (cross-modal attention fusion block).

Layout: feature-major activations on-chip — every tile is
[128 features (partitions) x 512 samples (free)], so all matmuls are
weight-stationary bf16 with the batch as the moving free dimension.
Inputs are pre-cast to bf16 host-side and loaded straight into
feature-major SBUF via DMA-transpose (2-byte xbar path), so no on-chip
input transposes are needed.

Cross-partition work (LayerNorm stats, softmax sums) is done with
ones-matrix matmuls that fuse the partition reduction AND the partition
broadcast into a single PE instruction.  The 2-way attention softmax
collapses to sigmoid((s0-s1)/sqrt(dh)), with s0-s1 accumulated in PSUM
by a +/- pair of block-diagonal head-mask matmuls.

LayerNorm mean subtraction is folded into the weights host-side
(centering matrix C = I - 11^T/128 on each producing linear layer);
the residual-stream means are zero by construction given the
(asserted) unit/zero LN affine params.

Data parallel over 8 NeuronCores: 8192 samples each.
"""

import numpy as np
import ml_dtypes

import concourse.bass as bass
import concourse.mybir as mybir
from concourse.tile import TileContext
from concourse.vector_clock import ScopedClock
from concourse.bass_utils import run_bass_kernel_spmd

F32 = mybir.dt.float32
BF16 = mybir.dt.bfloat16
AL = mybir.AluOpType
AF = mybir.ActivationFunctionType
NPBF = ml_dtypes.bfloat16

D = 128
SP = 1280
FFN = 256
NB = 3
DH = 32
KV_IDX = ((1, 2), (0, 2), (0, 1))
NCORES = 8
BLK = 1024
MMN = 512
EPS = 1e-5
ISQ = float(1.0 / np.sqrt(DH))


def _patch_tile_drain():
    """walrus here rejects >4 sem waits on one instruction; Tile's tail
    drain carries one wait per logical proc.  Re-emit them as standalone
    wait_ge instructions ahead of the drain."""
    TC = TileContext
    if getattr(TC, "_drain_patched", False):
        return

    def patched(self, tick_clock, wait_clock):
        nop_inst = self.nc.sync.nop()
        wait_clock.add_sem_waits(
            nop_inst.ins, ScopedClock({None: tick_clock.global_clock})
        )
        d = nop_inst.ins
        si = d.sync_info
        waits = list(si.on_wait) if si is not None else []
        if len(waits) > 4:
            si.on_wait = []
            d.sync_info = si
            name2sem = {s.name: s for s in self.sems.allocated().values()}
            for w in waits:
                sem = name2sem.get(w.ant_name)
                if sem is None:
                    raise RuntimeError(f"drain patch: unknown sem {w.ant_name}")
                self.nc.sync.wait_ge(sem, w.wait_value)
        self.nc.sync.drain()
        self.nc.all_engine_barrier()
        popped = self.nc._tile_sem_poison_stack.pop()
        assert popped is self._sem_poison
        self.nc.clear_and_free_semaphores(list(self.sems.allocated().values()))
        self.nc.all_engine_barrier()

    TC._drain_and_barrier = patched
    TC._drain_patched = True


def _fix_wait_overflow(nc):
    """walrus enforces per-opcode caps on sync-wait commands attached to
    one instruction (DmaTransposeAnt: 1, others: ~4).  Move the excess
    onto same-engine NOPs inserted immediately before the instruction."""
    LIMITS = {}
    DEFAULT_LIM = 1
    for fn in nc.m.functions:
        for bb in fn.blocks:
            insts = list(bb.instructions)
            out = []
            changed = False
            for inst in insts:
                si = getattr(inst, "sync_info", None)
                w = list(si.on_wait) if si is not None and si.on_wait else []
                lim = LIMITS.get(type(inst).__name__, DEFAULT_LIM)
                if len(w) > lim:
                    excess = w[lim:]
                    keep = w[:lim]
                    eng = nc.engines[inst.engine]
                    nops = []
                    for i in range(0, len(excess), 1):
                        chunk = excess[i:i + 1]
                        nop_bi = eng.nop()
                        nop_inst = nop_bi.ins
                        cb = nc.cur_bb.bb
                        cb.instructions = [x for x in cb.instructions
                                           if x.name != nop_inst.name]
                        import bass_rust
                        nop_inst.sync_info = bass_rust.SyncInfo(
                            on_wait=chunk, on_update=[])
                        nops.append(nop_inst)
                    si.on_wait = keep
                    inst.sync_info = si
                    out.extend(nops)
                    changed = True
                out.append(inst)
            if changed:
                bb.instructions = out


def prep_weights(inp):
    """Host-side prep of all weights into SBUF layouts. bf16 for matmul
    operands, fp32 for per-partition bias vectors."""
    f64 = np.float64
    C = np.eye(D, dtype=f64) - 1.0 / D

    def bf(a):
        return np.ascontiguousarray(a.astype(np.float32)).astype(NPBF)

    def f32(a):
        return np.ascontiguousarray(a, dtype=np.float32)

    w = {}
    wsp = C @ inp["proj_w_spatial"].astype(f64)            # [128,1280]
    w["wspT"] = bf(np.transpose(wsp.reshape(D, 10, D), (2, 1, 0)).reshape(D, 10 * D))
    wgf = np.stack([C @ inp["proj_w_gf"][i].astype(f64) for i in range(2)])
    w["wgfT"] = bf(np.transpose(wgf, (2, 0, 1)).reshape(D, 2 * D))
    w["bc"] = f32(C @ inp["proj_b"].astype(f64).T)         # [128,3]
    w["emb"] = f32(inp["mod_emb"].T)

    ipw = inp["in_proj_w"].astype(f64)                     # [3, 384, 128]
    wq, wk, wv = ipw[:, :D], ipw[:, D:2 * D], ipw[:, 2 * D:]
    w["wqT"] = bf(np.transpose(wq, (2, 0, 1)).reshape(D, NB * D))
    w["wkT"] = bf(np.transpose(wk, (2, 0, 1)).reshape(D, NB * D))
    w["wvT"] = bf(np.transpose(wv, (2, 0, 1)).reshape(D, NB * D))
    ow = np.stack([C @ inp["out_proj_w"][n].astype(f64) for n in range(NB)])
    w["owT"] = bf(np.transpose(ow, (2, 0, 1)).reshape(D, NB * D))
    ob2 = np.stack([
        C @ inp["out_proj_b"][n].astype(f64)
        - inp["mod_emb"][n].astype(f64).mean()
        for n in range(NB)])
    w["ob2"] = f32(ob2.T)

    w1 = inp["ffn_w1"].astype(f64)                         # [3, 256, 128]
    w["w1T"] = bf(np.transpose(w1, (2, 0, 1)).reshape(D, NB * FFN))
    w["b1"] = f32(inp["ffn_b1"].reshape(NB * 2, D).T)      # [128, 6]
    w2 = np.stack([C @ inp["ffn_w2"][n].astype(f64) for n in range(NB)])
    w2c = w2.reshape(NB, D, 2, D)                          # [n, j, c, p]
    w["w2T"] = bf(np.transpose(w2c, (3, 0, 2, 1)).reshape(D, NB * 2 * D))
    b2c = np.stack([C @ inp["ffn_b2"][n].astype(f64) for n in range(NB)])
    w["b2c"] = f32(b2c.T)

    gw = inp["gate_w"].astype(f64).reshape(NB, NB, D)      # [j, n, p]
    w["gwT"] = bf(np.transpose(gw, (2, 1, 0)).reshape(D, NB * NB))
    w["gateb"] = f32(inp["gate_b"].reshape(NB, 1))

    w["onesT"] = bf(np.full((D, D), 1.0 / D))
    hs = np.zeros((D, D), dtype=np.float32)
    for h in range(4):
        hs[h * DH:(h + 1) * DH, h * DH:(h + 1) * DH] = 1.0
    w["hsel"] = bf(hs)
    w["hseln"] = bf(-hs)
    w["ones3"] = bf(np.ones((NB, D)))
    esel = np.zeros((NB, NB * D), dtype=np.float32)
    for n in range(NB):
        esel[n, n * D:(n + 1) * D] = 1.0
    w["esel"] = bf(esel)
    w["ident"] = bf(np.eye(D))
    w["epsv"] = np.full((D, 1), EPS, dtype=np.float32)
    w["zerov"] = np.zeros((D, 1), dtype=np.float32)

    assert np.allclose(inp["proj_ln_g"], 1) and np.allclose(inp["proj_ln_b"], 0)
    assert np.allclose(inp["attn_ln_g"], 1) and np.allclose(inp["attn_ln_b"], 0)
    assert np.allclose(inp["ffn_ln_g"], 1) and np.allclose(inp["ffn_ln_b"], 0)
    assert np.allclose(inp["in_proj_b"], 0)
    return w


WEIGHT_SPECS = {
    "wspT": ((D, 10 * D), BF16), "wgfT": ((D, 2 * D), BF16),
    "bc": ((D, NB), F32), "emb": ((D, NB), F32),
    "wqT": ((D, NB * D), BF16), "wkT": ((D, NB * D), BF16),
    "wvT": ((D, NB * D), BF16), "owT": ((D, NB * D), BF16),
    "ob2": ((D, NB), F32),
    "w1T": ((D, NB * FFN), BF16), "b1": ((D, NB * 2), F32),
    "w2T": ((D, NB * 2 * D), BF16), "b2c": ((D, NB), F32),
    "gwT": ((D, NB * NB), BF16), "gateb": ((NB, 1), F32),
    "onesT": ((D, D), BF16), "hsel": ((D, D), BF16), "hseln": ((D, D), BF16),
    "ones3": ((NB, D), BF16), "esel": ((NB, NB * D), BF16),
    "ident": ((D, D), BF16),
    "epsv": ((D, 1), F32), "zerov": ((D, 1), F32),
}


def build_program(Bc, repeat=1):
    nc = bass.Bass()
    xsp = nc.dram_tensor("x_spatial", [Bc, SP], BF16, kind="ExternalInput")
    xg = nc.dram_tensor("x_gradient", [Bc, D], BF16, kind="ExternalInput")
    xf = nc.dram_tensor("x_frequency", [Bc, D], BF16, kind="ExternalInput")
    wd = {k: nc.dram_tensor(k, list(s[0]), s[1], kind="ExternalInput")
          for k, s in WEIGHT_SPECS.items()}
    out = nc.dram_tensor("outT", [D, Bc], BF16, kind="ExternalOutput")

    nblk = Bc // BLK
    assert Bc % BLK == 0

    with TileContext(nc) as tc, nc.allow_low_precision(reason="bf16 kernel"):
        with (
            tc.tile_pool(name="wp", bufs=1) as wp,
            tc.tile_pool(name="xin", bufs=2) as xin,
            tc.tile_pool(name="work", bufs=2) as wk_,
            tc.tile_pool(name="outp", bufs=1) as outp,
            tc.tile_pool(name="ps", bufs=4, space="PSUM") as psp,
        ):
            W = {}
            for k, s in WEIGHT_SPECS.items():
                W[k] = wp.tile(list(s[0]), s[1], tag=k, name=k)
                nc.gpsimd.dma_start(W[k][:], wd[k][:])
            ident = W["ident"]

            def mm(out_ap, lhsT, rhs, start=True, stop=True):
                for h in range(BLK // MMN):
                    nc.tensor.matmul(out_ap[:, h * MMN:(h + 1) * MMN], lhsT,
                                     rhs[:, h * MMN:(h + 1) * MMN],
                                     start=start, stop=stop)

            def phase0(b):
                r0 = (b % nblk) * BLK
                st = {}
                xspT_all = xin.tile([D, 10 * BLK], BF16, tag="xspT")
                nc.sync.dma_start(
                    xspT_all[:].rearrange("p (c n) -> p c n", c=10),
                    xsp[r0:r0 + BLK, :], transpose=True)
                st["xspT"] = xspT_all
                st["xgT"] = xin.tile([D, BLK], BF16, tag="xgT", name="xgT")
                nc.sync.dma_start(st["xgT"][:], xg[r0:r0 + BLK, :], transpose=True)
                st["xfT"] = xin.tile([D, BLK], BF16, tag="xfT", name="xfT")
                nc.sync.dma_start(st["xfT"][:], xf[r0:r0 + BLK, :], transpose=True)
                return st

            def ln_rb_into(sq_sb, rb):
                mq = psp.tile([D, BLK], F32, tag="ps")
                mm(mq[:], W["onesT"][:], sq_sb[:])
                lnv = wk_.tile([D, BLK], F32, tag="lnv", bufs=1)
                nc.scalar.activation(lnv[:], mq[:], AF.Ln,
                                     bias=W["epsv"][:, 0:1])
                nc.scalar.activation(rb[:], lnv[:], AF.Exp, scale=-0.5,
                                     bias=W["zerov"][:, 0:1])

            def ln_rb(sq_sb, tag):
                # rsqrt(v+eps) = exp(-0.5*ln(v+eps)): stays in the
                # natural_log_exp table set (no ACT table switches)
                mq = psp.tile([D, BLK], F32, tag="ps")
                mm(mq[:], W["onesT"][:], sq_sb[:])
                lnv = wk_.tile([D, BLK], F32, tag="lnv", bufs=1)
                nc.scalar.activation(lnv[:], mq[:], AF.Ln,
                                     bias=W["epsv"][:, 0:1])
                rb = wk_.tile([D, BLK], BF16, tag=tag)
                nc.scalar.activation(rb[:], lnv[:], AF.Exp, scale=-0.5,
                                     bias=W["zerov"][:, 0:1])
                return rb

            def phase1(st):
                z_ps = []
                zs = psp.tile([D, BLK], F32, tag="ps")
                for c in range(10):
                    mm(zs[:], W["wspT"][:, c * D:(c + 1) * D],
                       st["xspT"][:, c * BLK:(c + 1) * BLK],
                       start=(c == 0), stop=(c == 9))
                z_ps.append(zs)
                for i, key in ((0, "xgT"), (1, "xfT")):
                    zt = psp.tile([D, BLK], F32, tag="ps")
                    mm(zt[:], W["wgfT"][:, i * D:(i + 1) * D], st[key][:])
                    z_ps.append(zt)
                P = []
                for n in range(NB):
                    z_sb = wk_.tile([D, BLK], BF16, tag=f"zsb{n}", bufs=1)
                    nc.scalar.activation(z_sb[:], z_ps[n][:], AF.Identity,
                                         bias=W["bc"][:, n:n + 1])
                    sq = wk_.tile([D, BLK], BF16, tag="sq", bufs=1)
                    nc.scalar.activation(sq[:], z_sb[:], AF.Square,
                                         bias=W["zerov"][:, 0:1])
                    rb = ln_rb(sq, "rb")
                    p_ = wk_.tile([D, BLK], BF16, tag=f"P{n}")
                    nc.vector.tensor_tensor(p_[:], z_sb[:], rb[:], AL.mult)
                    nc.vector.tensor_scalar_add(p_[:], p_[:], W["emb"][:, n:n + 1])
                    P.append(p_)
                st["P"] = P
                # dP[n] = P[kv0] - P[kv1]: k/v differences come from a single
                # matmul each (linearity), halving attention PSUM pressure
                dP = []
                for n in range(NB):
                    s0, s1 = KV_IDX[n]
                    dp = wk_.tile([D, BLK], BF16, tag=f"dP{n}", bufs=2)
                    nc.vector.tensor_tensor(dp[:], P[s0][:], P[s1][:], AL.subtract)
                    dP.append(dp)
                st["dP"] = dP

            def phase2(st):
                P = st["P"]
                dP = st["dP"]
                x1 = []
                for n in range(NB):
                    s0, s1 = KV_IDX[n]
                    q_ps = psp.tile([D, BLK], F32, tag="ps")
                    mm(q_ps[:], W["wqT"][:, n * D:(n + 1) * D], P[n][:])
                    dk_ps = psp.tile([D, BLK], F32, tag="ps")
                    mm(dk_ps[:], W["wkT"][:, n * D:(n + 1) * D], dP[n][:])
                    dv_ps = psp.tile([D, BLK], F32, tag="ps")
                    mm(dv_ps[:], W["wvT"][:, n * D:(n + 1) * D], dP[n][:])
                    v1_ps = psp.tile([D, BLK], F32, tag="ps")
                    mm(v1_ps[:], W["wvT"][:, n * D:(n + 1) * D], P[s1][:])

                    q_sb = wk_.tile([D, BLK], BF16, tag="qsb", bufs=1)
                    nc.scalar.activation(q_sb[:], q_ps[:], AF.Copy)
                    t0 = wk_.tile([D, BLK], BF16, tag="t0", bufs=1)
                    nc.vector.tensor_tensor(t0[:], q_sb[:], dk_ps[:], AL.mult)
                    d_ps = psp.tile([D, BLK], F32, tag="ps")
                    mm(d_ps[:], W["hsel"][:], t0[:])
                    # sigmoid(d*ISQ) = 1/(1+exp(-d*ISQ)) — ln_exp set only
                    ea = wk_.tile([D, BLK], BF16, tag="ea", bufs=1)
                    nc.scalar.activation(ea[:], d_ps[:], AF.Exp,
                                         bias=W["zerov"][:, 0:1], scale=-ISQ)
                    ea1 = wk_.tile([D, BLK], BF16, tag="ea1", bufs=1)
                    nc.vector.tensor_scalar_add(ea1[:], ea[:], 1.0)
                    a0 = wk_.tile([D, BLK], BF16, tag="a0", bufs=1)
                    nc.vector.reciprocal(a0[:], ea1[:])
                    tp = wk_.tile([D, BLK], BF16, tag="tp", bufs=1)
                    nc.vector.tensor_tensor(tp[:], a0[:], dv_ps[:], AL.mult)
                    tpv = wk_.tile([D, BLK], BF16, tag="tpv", bufs=1)
                    nc.vector.tensor_tensor(tpv[:], tp[:], v1_ps[:], AL.add)

                    o_ps = psp.tile([D, BLK], F32, tag="ps")
                    mm(o_ps[:], W["owT"][:, n * D:(n + 1) * D], tpv[:])
                    u = wk_.tile([D, BLK], BF16, tag=f"u{n}")
                    nc.vector.scalar_tensor_tensor(
                        u[:], o_ps[:], W["ob2"][:, n:n + 1], P[n][:],
                        AL.add, AL.add)
                    sq = wk_.tile([D, BLK], BF16, tag="sq", bufs=1)
                    nc.scalar.activation(sq[:], u[:], AF.Square,
                                         bias=W["zerov"][:, 0:1])
                    rb = wk_.tile([D, BLK], BF16, tag=f"rb2_{n}")
                    ln_rb_into(sq, rb)
                    x1n = wk_.tile([D, BLK], BF16, tag=f"x1{n}")
                    nc.vector.tensor_tensor(x1n[:], u[:], rb[:], AL.mult)
                    x1.append(x1n)
                    st.setdefault("u", []).append(u)
                    st.setdefault("rb2", []).append(rb)
                st["x1"] = x1

            def phase3a(st):
                u, rb2 = st["u"], st["rb2"]
                hs_all = []
                for n in range(NB):
                    h_sb = []
                    for c in range(2):
                        h_ps = psp.tile([D, BLK], F32, tag="ps")
                        mm(h_ps[:],
                           W["w1T"][:, n * FFN + c * D: n * FFN + (c + 1) * D],
                           u[n][:])
                        hpre = wk_.tile([D, BLK], BF16, tag=f"hpre{c}", bufs=1)
                        nc.vector.tensor_tensor(hpre[:], rb2[n][:], h_ps[:],
                                                AL.mult)
                        hs_ = wk_.tile([D, BLK], BF16, tag=f"hsb{n}_{c}", bufs=1)
                        nc.scalar.activation(hs_[:], hpre[:], AF.Gelu,
                                             bias=W["b1"][:, 2 * n + c: 2 * n + c + 1])
                        h_sb.append(hs_)
                    hs_all.append(h_sb)
                st["hs"] = hs_all

            def phase3b(st):
                x1 = st["x1"]
                x2 = []
                for n in range(NB):
                    h_sb = st["hs"][n]
                    f_ps = psp.tile([D, BLK], F32, tag="ps")
                    for c in range(2):
                        mm(f_ps[:], W["w2T"][:, (2 * n + c) * D:(2 * n + c + 1) * D],
                           h_sb[c][:], start=(c == 0), stop=(c == 1))
                    x2p = wk_.tile([D, BLK], BF16, tag="x2p", bufs=1)
                    nc.vector.scalar_tensor_tensor(
                        x2p[:], f_ps[:], W["b2c"][:, n:n + 1], x1[n][:],
                        AL.add, AL.add)
                    sq = wk_.tile([D, BLK], BF16, tag="sq", bufs=1)
                    nc.scalar.activation(sq[:], x2p[:], AF.Square,
                                         bias=W["zerov"][:, 0:1])
                    rb = ln_rb(sq, "rb")
                    x2n = wk_.tile([D, BLK], BF16, tag=f"x2{n}")
                    nc.vector.tensor_tensor(x2n[:], x2p[:], rb[:], AL.mult)
                    x2.append(x2n)
                st["x2"] = x2

            def phase4a(st):
                x2 = st["x2"]
                g_ps = psp.tile([NB, BLK], F32, tag="ps")
                for n in range(NB):
                    mm(g_ps[:], W["gwT"][:, n * NB:(n + 1) * NB], x2[n][:],
                       start=(n == 0), stop=(n == 2))
                e_sb = wk_.tile([NB, BLK], BF16, tag="esb", bufs=1)
                nc.scalar.activation(e_sb[:], g_ps[:], AF.Exp,
                                     bias=W["gateb"][:NB, 0:1])
                zb_ps = psp.tile([D, BLK], F32, tag="ps")
                mm(zb_ps[:], W["ones3"][:NB, :], e_sb[:])
                rz = wk_.tile([D, BLK], BF16, tag="rz", bufs=1)
                nc.vector.reciprocal(rz[:], zb_ps[:])
                mns = []
                for n in range(NB):
                    eb_ps = psp.tile([D, BLK], F32, tag="ps")
                    mm(eb_ps[:], W["esel"][:NB, n * D:(n + 1) * D], e_sb[:])
                    mn = wk_.tile([D, BLK], BF16, tag=f"mn{n}", bufs=2)
                    nc.vector.tensor_tensor(mn[:], x2[n][:], eb_ps[:], AL.mult)
                    mns.append(mn)
                st["mn"] = mns
                st["rz"] = rz

            def phase4b(st, b):
                r0 = (b % nblk) * BLK
                mns, rz = st["mn"], st["rz"]
                acc = wk_.tile([D, BLK], BF16, tag="macc", bufs=1)
                nc.vector.tensor_tensor(acc[:], mns[0][:], mns[1][:], AL.add)
                acc2 = wk_.tile([D, BLK], BF16, tag="macc2", bufs=1)
                nc.vector.tensor_tensor(acc2[:], acc[:], mns[2][:], AL.add)
                fused = wk_.tile([D, BLK], BF16, tag="fused", bufs=1)
                nc.vector.tensor_tensor(fused[:], acc2[:], rz[:], AL.mult)
                nc.gpsimd.dma_start(out[:, r0:r0 + BLK], fused[:])

            # software-pipelined emission; gelu ops grouped at tick head so
            # the ACT table set switches at most twice per tick
            total = nblk * repeat
            bstate = {}
            for t in range(total + 4):
                if 0 <= t - 3 < total:
                    phase3a(bstate[t - 3])
                if 0 <= t - 4 < total:
                    phase4a(bstate[t - 4])
                if 0 <= t - 2 < total:
                    phase2(bstate[t - 2])
                if 0 <= t - 1 < total:
                    phase1(bstate[t - 1])
                if 0 <= t - 3 < total:
                    phase3b(bstate[t - 3])
                if 0 <= t - 4 < total:
                    phase4b(bstate.pop(t - 4), t - 4)
                if t < total:
                    bstate[t] = phase0(t)
    _fix_wait_overflow(nc)
    return nc


def kernel(**inputs):
    _patch_tile_drain()
    B = inputs["x_spatial"].shape[0]
    Bc = B // NCORES
    w = prep_weights(inputs)
    nc = build_program(Bc)
    xb = {k: np.ascontiguousarray(inputs[k]).astype(NPBF)
          for k in ("x_spatial", "x_gradient", "x_frequency")}
    in_maps = []
    for c in range(NCORES):
        m = dict(w)
        for k in ("x_spatial", "x_gradient", "x_frequency"):
            m[k] = np.ascontiguousarray(xb[k][c * Bc:(c + 1) * Bc])
        in_maps.append(m)
    res = run_bass_kernel_spmd(nc, in_maps, list(range(NCORES)))
    outs = [res.results[c]["outT"] for c in range(NCORES)]
    full = np.concatenate([o.T for o in outs], axis=0)
    return np.ascontiguousarray(full.astype(np.float32))



# revision 15
# speedup vs baseline: 1.4063x; 1.4063x over previous
"""Trainium2 Bass kernel for nn_CMAF (cross-modal attention fusion block).

Layout: feature-major activations on-chip — every tile is
[128 features (partitions) x 1024 samples (free)], so all matmuls are
weight-stationary bf16 with the batch as the moving free dimension.
Inputs are pre-transposed host-side into feature-major HBM layouts, so
device DMA is fully contiguous (no DMA-transpose).

Engine-balance design (ACT/DVE were the baseline bottleneck):
 - LN stats (sum of squares) for all 3 branches land in ONE [3,1024]
   PSUM tile; Ln+Exp (rsqrt) run once per LN stage on that compact tile
   instead of per-branch full tiles; per-branch ones-matmuls broadcast
   the result back to 128 partitions (PE pump is cheap).
 - Residual adds (u = o + P, x2p = f + x1) are folded into the PE as
   identity-matrix accumulation matmuls, killing 1x-rate STT DVE ops.
 - Wo@v1 is folded host-side into Wov = (C Wo) Wv and accumulated into
   the same PSUM as Wo@tp, killing the tpv add.
 - The 2-way attention softmax collapses to division by (1+exp(-d/sqrt(dh)))
   done as a single DVE tensor_tensor divide straight from PSUM.
 - Gelu ACT ops are clustered at alternating head/tail of the pipeline
   tick so the ACT table set (gelu vs natural_log_exp) switches once per
   block on average instead of twice.

Data parallel over 8 NeuronCores: 8192 samples each.
"""

import numpy as np
import ml_dtypes

import concourse.bass as bass
import concourse.mybir as mybir
from concourse.tile import TileContext
from concourse.vector_clock import ScopedClock
from concourse.bass_utils import run_bass_kernel_spmd

F32 = mybir.dt.float32
BF16 = mybir.dt.bfloat16
AL = mybir.AluOpType
AF = mybir.ActivationFunctionType
NPBF = ml_dtypes.bfloat16

D = 128
SP = 1280
FFN = 256
NB = 3
DH = 32
KV_IDX = ((1, 2), (0, 2), (0, 1))
NCORES = 8
BLK = 1024
MMN = 512
EPS = 1e-5
ISQ = float(1.0 / np.sqrt(DH))

# tuning flags
IDENT_FOLD = True      # residual adds via identity matmuls on PE


def _patch_tile_drain():
    """walrus here rejects >4 sem waits on one instruction; Tile's tail
    drain carries one wait per logical proc.  Re-emit them as standalone
    wait_ge instructions ahead of the drain."""
    TC = TileContext
    if getattr(TC, "_drain_patched", False):
        return

    def patched(self, tick_clock, wait_clock):
        nop_inst = self.nc.sync.nop()
        wait_clock.add_sem_waits(
            nop_inst.ins, ScopedClock({None: tick_clock.global_clock})
        )
        d = nop_inst.ins
        si = d.sync_info
        waits = list(si.on_wait) if si is not None else []
        if len(waits) > 4:
            si.on_wait = []
            d.sync_info = si
            name2sem = {s.name: s for s in self.sems.allocated().values()}
            for w in waits:
                sem = name2sem.get(w.ant_name)
                if sem is None:
                    raise RuntimeError(f"drain patch: unknown sem {w.ant_name}")
                self.nc.sync.wait_ge(sem, w.wait_value)
        self.nc.sync.drain()
        self.nc.all_engine_barrier()
        popped = self.nc._tile_sem_poison_stack.pop()
        assert popped is self._sem_poison
        self.nc.clear_and_free_semaphores(list(self.sems.allocated().values()))
        self.nc.all_engine_barrier()

    TC._drain_and_barrier = patched
    TC._drain_patched = True


def _fix_wait_overflow(nc):
    """walrus enforces per-opcode caps on sync-wait commands attached to
    one instruction (DmaTransposeAnt: 1, others: ~4).  Move the excess
    onto same-engine NOPs inserted immediately before the instruction."""
    LIMITS = {}
    DEFAULT_LIM = 1
    for fn in nc.m.functions:
        for bb in fn.blocks:
            insts = list(bb.instructions)
            out = []
            changed = False
            for inst in insts:
                si = getattr(inst, "sync_info", None)
                w = list(si.on_wait) if si is not None and si.on_wait else []
                lim = LIMITS.get(type(inst).__name__, DEFAULT_LIM)
                if len(w) > lim:
                    excess = w[lim:]
                    keep = w[:lim]
                    eng = nc.engines[inst.engine]
                    nops = []
                    for i in range(0, len(excess), 1):
                        chunk = excess[i:i + 1]
                        nop_bi = eng.nop()
                        nop_inst = nop_bi.ins
                        cb = nc.cur_bb.bb
                        cb.instructions = [x for x in cb.instructions
                                           if x.name != nop_inst.name]
                        import bass_rust
                        nop_inst.sync_info = bass_rust.SyncInfo(
                            on_wait=chunk, on_update=[])
                        nops.append(nop_inst)
                    si.on_wait = keep
                    inst.sync_info = si
                    out.extend(nops)
                    changed = True
                out.append(inst)
            if changed:
                bb.instructions = out


def prep_weights(inp):
    """Host-side prep of all weights into SBUF layouts. bf16 for matmul
    operands, fp32 for per-partition bias vectors."""
    f64 = np.float64
    C = np.eye(D, dtype=f64) - 1.0 / D

    def bf(a):
        return np.ascontiguousarray(a.astype(np.float32)).astype(NPBF)

    def f32(a):
        return np.ascontiguousarray(a, dtype=np.float32)

    w = {}
    wsp = C @ inp["proj_w_spatial"].astype(f64)            # [128,1280]
    w["wspT"] = bf(np.transpose(wsp.reshape(D, 10, D), (2, 1, 0)).reshape(D, 10 * D))
    wgf = np.stack([C @ inp["proj_w_gf"][i].astype(f64) for i in range(2)])
    w["wgfT"] = bf(np.transpose(wgf, (2, 0, 1)).reshape(D, 2 * D))
    w["bc"] = f32(C @ inp["proj_b"].astype(f64).T)         # [128,3]
    w["emb"] = f32(inp["mod_emb"].T)

    ipw = inp["in_proj_w"].astype(f64)                     # [3, 384, 128]
    wq, wk, wv = ipw[:, :D], ipw[:, D:2 * D], ipw[:, 2 * D:]
    w["wqT"] = bf(np.transpose(wq, (2, 0, 1)).reshape(D, NB * D))
    w["wkT"] = bf(np.transpose(wk, (2, 0, 1)).reshape(D, NB * D))
    w["wvT"] = bf(np.transpose(wv, (2, 0, 1)).reshape(D, NB * D))
    ow = np.stack([C @ inp["out_proj_w"][n].astype(f64) for n in range(NB)])
    # 0.5x fold: attention prob a = (1+tanh(d/(2 sqrt(dh))))/2, the 1/2 is
    # folded here so tp = (tanh+1)*dv feeds Wo directly
    w["owT"] = bf(0.5 * np.transpose(ow, (2, 0, 1)).reshape(D, NB * D))
    # Wov[n] = (C @ Wo[n]) @ Wv[n] : folds the v1 path into one matmul
    wov = np.stack([ow[n] @ wv[n] for n in range(NB)])
    w["wovT"] = bf(np.transpose(wov, (2, 0, 1)).reshape(D, NB * D))
    ob2 = np.stack([
        C @ inp["out_proj_b"][n].astype(f64)
        - inp["mod_emb"][n].astype(f64).mean()
        for n in range(NB)])
    w["ob2"] = f32(ob2.T)

    w1 = inp["ffn_w1"].astype(f64)                         # [3, 256, 128]
    w["w1T"] = bf(np.transpose(w1, (2, 0, 1)).reshape(D, NB * FFN))
    w["b1"] = f32(inp["ffn_b1"].reshape(NB * 2, D).T)      # [128, 6]
    w2 = np.stack([C @ inp["ffn_w2"][n].astype(f64) for n in range(NB)])
    w2c = w2.reshape(NB, D, 2, D)                          # [n, j, c, p]
    w["w2T"] = bf(np.transpose(w2c, (3, 0, 2, 1)).reshape(D, NB * 2 * D))
    b2c = np.stack([C @ inp["ffn_b2"][n].astype(f64) for n in range(NB)])
    w["b2c"] = f32(b2c.T)

    gw = inp["gate_w"].astype(f64).reshape(NB, NB, D)      # [j, n, p]
    w["gwT"] = bf(np.transpose(gw, (2, 1, 0)).reshape(D, NB * NB))
    w["gateb"] = f32(inp["gate_b"].reshape(NB, 1))

    w["onesT"] = bf(np.full((D, D), 1.0 / D))
    svsel = np.zeros((D, NB * NB), dtype=np.float32)
    for n in range(NB):
        svsel[:, NB * n + n] = 1.0 / D
    w["svsel"] = bf(svsel)
    hs = np.zeros((D, D), dtype=np.float32)
    for h in range(4):
        hs[h * DH:(h + 1) * DH, h * DH:(h + 1) * DH] = 1.0
    w["hsel"] = bf(hs)
    w["ones3"] = bf(np.ones((NB, D)))
    esel = np.zeros((NB, NB * D), dtype=np.float32)
    for n in range(NB):
        esel[n, n * D:(n + 1) * D] = 1.0
    w["esel"] = bf(esel)
    w["ident"] = bf(np.eye(D))
    w["epsv"] = np.full((D, 1), EPS, dtype=np.float32)
    w["zerov"] = np.zeros((D, 1), dtype=np.float32)

    assert np.allclose(inp["proj_ln_g"], 1) and np.allclose(inp["proj_ln_b"], 0)
    assert np.allclose(inp["attn_ln_g"], 1) and np.allclose(inp["attn_ln_b"], 0)
    assert np.allclose(inp["ffn_ln_g"], 1) and np.allclose(inp["ffn_ln_b"], 0)
    assert np.allclose(inp["in_proj_b"], 0)
    return w


WEIGHT_SPECS = {
    "wspT": ((D, 10 * D), BF16), "wgfT": ((D, 2 * D), BF16),
    "bc": ((D, NB), F32), "emb": ((D, NB), F32),
    "wqT": ((D, NB * D), BF16), "wkT": ((D, NB * D), BF16),
    "wvT": ((D, NB * D), BF16), "owT": ((D, NB * D), BF16),
    "wovT": ((D, NB * D), BF16),
    "ob2": ((D, NB), F32),
    "w1T": ((D, NB * FFN), BF16), "b1": ((D, NB * 2), F32),
    "w2T": ((D, NB * 2 * D), BF16), "b2c": ((D, NB), F32),
    "gwT": ((D, NB * NB), BF16), "gateb": ((NB, 1), F32),
    "onesT": ((D, D), BF16), "hsel": ((D, D), BF16),
    "svsel": ((D, NB * NB), BF16),
    "ones3": ((NB, D), BF16), "esel": ((NB, NB * D), BF16),
    "ident": ((D, D), BF16),
    "epsv": ((D, 1), F32), "zerov": ((D, 1), F32),
}


def build_program(Bc, repeat=1):
    nc = bass.Bass()
    # pre-transposed feature-major inputs in HBM
    xsp = nc.dram_tensor("xspT", [D, 10, Bc], BF16, kind="ExternalInput")
    xg = nc.dram_tensor("xgT", [D, Bc], BF16, kind="ExternalInput")
    xf = nc.dram_tensor("xfT", [D, Bc], BF16, kind="ExternalInput")
    wd = {k: nc.dram_tensor(k, list(s[0]), s[1], kind="ExternalInput")
          for k, s in WEIGHT_SPECS.items()}
    out = nc.dram_tensor("outT", [D, Bc], BF16, kind="ExternalOutput")

    nblk = Bc // BLK
    assert Bc % BLK == 0

    with TileContext(nc) as tc, nc.allow_low_precision(reason="bf16 kernel"):
        with (
            tc.tile_pool(name="wp", bufs=1) as wp,
            tc.tile_pool(name="xin", bufs=2) as xin,
            tc.tile_pool(name="work", bufs=2) as wk_,
            tc.tile_pool(name="ps", bufs=3, space="PSUM") as psp,
        ):
            W = {}
            for k, s in WEIGHT_SPECS.items():
                W[k] = wp.tile(list(s[0]), s[1], tag=k, name=k)
                nc.gpsimd.dma_start(W[k][:], wd[k][:])
            ident = W["ident"]

            def mm(out_ap, lhsT, rhs, start=True, stop=True):
                for h in range(BLK // MMN):
                    nc.tensor.matmul(out_ap[:, h * MMN:(h + 1) * MMN], lhsT,
                                     rhs[:, h * MMN:(h + 1) * MMN],
                                     start=start, stop=stop)

            def phase0(b):
                r0 = (b % nblk) * BLK
                st = {}
                xspT_all = xin.tile([D, 10 * BLK], BF16, tag="xspT")
                nc.sync.dma_start(
                    xspT_all[:].rearrange("p (c n) -> p c n", c=10),
                    xsp[:, :, r0:r0 + BLK])
                st["xspT"] = xspT_all
                st["xgT"] = xin.tile([D, BLK], BF16, tag="xgT", name="xgT")
                nc.sync.dma_start(st["xgT"][:], xg[:, r0:r0 + BLK])
                st["xfT"] = xin.tile([D, BLK], BF16, tag="xfT", name="xfT")
                nc.sync.dma_start(st["xfT"][:], xf[:, r0:r0 + BLK])
                return st

            def compact_rsqrt(sv_ps, tag):
                """sv_ps: [NB, BLK] PSUM of per-branch mean-squares.
                Returns [NB, BLK] bf16 SBUF tile of rsqrt(v+eps)."""
                lnv = wk_.tile([NB, BLK], F32, tag="lnv", bufs=2)
                nc.scalar.activation(lnv[:], sv_ps[:], AF.Ln,
                                     bias=W["epsv"][:NB, 0:1])
                rbc = wk_.tile([NB, BLK], BF16, tag="rbc", bufs=3)
                nc.scalar.activation(rbc[:], lnv[:], AF.Exp, scale=-0.5,
                                     bias=W["zerov"][:NB, 0:1])
                return rbc

            def bcast(rbc, n):
                """broadcast row n of [NB, BLK] tile to [128, BLK] PSUM."""
                rbb = psp.tile([D, BLK], F32, tag="ps")
                mm(rbb[:], W["esel"][:NB, n * D:(n + 1) * D], rbc[:])
                return rbb

            def phase1(st):
                z_ps = []
                zs = psp.tile([D, BLK], F32, tag="ps")
                for c in range(10):
                    mm(zs[:], W["wspT"][:, c * D:(c + 1) * D],
                       st["xspT"][:, c * BLK:(c + 1) * BLK],
                       start=(c == 0), stop=(c == 9))
                z_ps.append(zs)
                for i, key in ((0, "xgT"), (1, "xfT")):
                    zt = psp.tile([D, BLK], F32, tag="ps")
                    mm(zt[:], W["wgfT"][:, i * D:(i + 1) * D], st[key][:])
                    z_ps.append(zt)
                zsb, sq = [], []
                for n in range(NB):
                    z_sb = wk_.tile([D, BLK], BF16, tag=f"zsb{n}", bufs=1)
                    nc.scalar.activation(z_sb[:], z_ps[n][:], AF.Identity,
                                         bias=W["bc"][:, n:n + 1])
                    zsb.append(z_sb)
                    s_ = wk_.tile([D, BLK], BF16, tag="sq1", bufs=1)
                    nc.vector.tensor_tensor(s_[:], z_sb[:], z_sb[:], AL.mult)
                    sq.append(s_)
                sv = psp.tile([NB, BLK], F32, tag="sv", bufs=1)
                for n in range(NB):
                    mm(sv[:], W["svsel"][:, NB * n:NB * (n + 1)], sq[n][:],
                       start=(n == 0), stop=(n == NB - 1))
                rbc = compact_rsqrt(sv, "1")
                P = []
                for n in range(NB):
                    rbb = bcast(rbc, n)
                    p0 = wk_.tile([D, BLK], BF16, tag="p0", bufs=1)
                    nc.vector.tensor_tensor(p0[:], zsb[n][:], rbb[:], AL.mult)
                    p_ = wk_.tile([D, BLK], BF16, tag=f"P{n}")
                    nc.vector.tensor_scalar_add(p_[:], p0[:], W["emb"][:, n:n + 1])
                    P.append(p_)
                st["P"] = P
                dP = []
                for n in range(NB):
                    s0, s1 = KV_IDX[n]
                    dp = wk_.tile([D, BLK], BF16, tag=f"dP{n}", bufs=2)
                    nc.vector.tensor_tensor(dp[:], P[s0][:], P[s1][:], AL.subtract)
                    dP.append(dp)
                st["dP"] = dP

            def phase2(st):
                P = st["P"]
                dP = st["dP"]
                us, sqs = [], []
                for n in range(NB):
                    s0, s1 = KV_IDX[n]
                    q_ps = psp.tile([D, BLK], F32, tag="ps")
                    mm(q_ps[:], W["wqT"][:, n * D:(n + 1) * D], P[n][:])
                    dk_ps = psp.tile([D, BLK], F32, tag="ps")
                    mm(dk_ps[:], W["wkT"][:, n * D:(n + 1) * D], dP[n][:])
                    dv_ps = psp.tile([D, BLK], F32, tag="ps")
                    mm(dv_ps[:], W["wvT"][:, n * D:(n + 1) * D], dP[n][:])

                    q_sb = wk_.tile([D, BLK], BF16, tag="qsb", bufs=1)
                    nc.scalar.activation(q_sb[:], q_ps[:], AF.Copy)
                    t0 = wk_.tile([D, BLK], BF16, tag="t0", bufs=1)
                    nc.vector.tensor_tensor(t0[:], q_sb[:], dk_ps[:], AL.mult)
                    d_ps = psp.tile([D, BLK], F32, tag="ps")
                    mm(d_ps[:], W["hsel"][:], t0[:])
                    # sigmoid via tanh (same ACT table set as gelu):
                    # a = (1+tanh(d*ISQ/2))/2, 1/2 folded into owT;
                    # tp = (tanh+1)*dv in one fused STT op
                    th = wk_.tile([D, BLK], BF16, tag="th", bufs=1)
                    nc.scalar.activation(th[:], d_ps[:], AF.Tanh,
                                         bias=W["zerov"][:, 0:1],
                                         scale=0.5 * ISQ)
                    tp = wk_.tile([D, BLK], BF16, tag="tp", bufs=1)
                    nc.vector.scalar_tensor_tensor(
                        tp[:], th[:], 1.0, dv_ps[:], AL.add, AL.mult)

                    o_ps = psp.tile([D, BLK], F32, tag="ps")
                    mm(o_ps[:], W["owT"][:, n * D:(n + 1) * D], tp[:],
                       start=True, stop=False)
                    mm(o_ps[:], W["wovT"][:, n * D:(n + 1) * D], P[s1][:],
                       start=False, stop=not IDENT_FOLD)
                    if IDENT_FOLD:
                        mm(o_ps[:], ident[:], P[n][:], start=False, stop=True)
                        u = wk_.tile([D, BLK], BF16, tag=f"u{n}", bufs=1)
                        nc.scalar.activation(u[:], o_ps[:], AF.Identity,
                                             bias=W["ob2"][:, n:n + 1])
                    else:
                        u = wk_.tile([D, BLK], BF16, tag=f"u{n}", bufs=1)
                        nc.vector.scalar_tensor_tensor(
                            u[:], o_ps[:], W["ob2"][:, n:n + 1], P[n][:],
                            AL.add, AL.add)
                    us.append(u)
                    s_ = wk_.tile([D, BLK], BF16, tag="sq2", bufs=1)
                    nc.vector.tensor_tensor(s_[:], u[:], u[:], AL.mult)
                    sqs.append(s_)
                sv = psp.tile([NB, BLK], F32, tag="sv", bufs=1)
                for n in range(NB):
                    mm(sv[:], W["svsel"][:, NB * n:NB * (n + 1)], sqs[n][:],
                       start=(n == 0), stop=(n == NB - 1))
                rbc = compact_rsqrt(sv, "2")
                x1 = []
                for n in range(NB):
                    rbb = bcast(rbc, n)
                    # x1 lives 3 ticks: made in p2(t-2), read by p3a(t-3)
                    # and p3b(t-4)
                    x1n = wk_.tile([D, BLK], BF16, tag=f"x1{n}", bufs=3)
                    nc.vector.tensor_tensor(x1n[:], us[n][:], rbb[:], AL.mult)
                    x1.append(x1n)
                st["x1"] = x1

            def phase3a(st):
                """FFN first half: W1 matmuls + gelu cluster."""
                x1 = st["x1"]
                hs_all = []
                for n in range(NB):
                    h_sb = []
                    for c in range(2):
                        h_ps = psp.tile([D, BLK], F32, tag="ps")
                        mm(h_ps[:],
                           W["w1T"][:, n * FFN + c * D: n * FFN + (c + 1) * D],
                           x1[n][:])
                        hs_ = wk_.tile([D, BLK], BF16, tag=f"hsb{n}_{c}", bufs=2)
                        nc.scalar.activation(hs_[:], h_ps[:], AF.Gelu,
                                             bias=W["b1"][:, 2 * n + c: 2 * n + c + 1])
                        h_sb.append(hs_)
                    hs_all.append(h_sb)
                st["hs"] = hs_all

            def phase3b(st):
                x1 = st["x1"]
                x2ps, sqs = [], []
                for n in range(NB):
                    h_sb = st["hs"][n]
                    f_ps = psp.tile([D, BLK], F32, tag="ps")
                    for c in range(2):
                        mm(f_ps[:], W["w2T"][:, (2 * n + c) * D:(2 * n + c + 1) * D],
                           h_sb[c][:], start=(c == 0),
                           stop=(c == 1 and not IDENT_FOLD))
                    if IDENT_FOLD:
                        mm(f_ps[:], ident[:], x1[n][:], start=False, stop=True)
                        x2p = wk_.tile([D, BLK], BF16, tag=f"x2p{n}", bufs=1)
                        nc.scalar.activation(x2p[:], f_ps[:], AF.Identity,
                                             bias=W["b2c"][:, n:n + 1])
                    else:
                        x2p = wk_.tile([D, BLK], BF16, tag=f"x2p{n}", bufs=1)
                        nc.vector.scalar_tensor_tensor(
                            x2p[:], f_ps[:], W["b2c"][:, n:n + 1], x1[n][:],
                            AL.add, AL.add)
                    x2ps.append(x2p)
                    s_ = wk_.tile([D, BLK], BF16, tag="sq3", bufs=1)
                    nc.vector.tensor_tensor(s_[:], x2p[:], x2p[:], AL.mult)
                    sqs.append(s_)
                sv = psp.tile([NB, BLK], F32, tag="sv", bufs=1)
                for n in range(NB):
                    mm(sv[:], W["svsel"][:, NB * n:NB * (n + 1)], sqs[n][:],
                       start=(n == 0), stop=(n == NB - 1))
                rbc = compact_rsqrt(sv, "3")
                x2 = []
                for n in range(NB):
                    rbb = bcast(rbc, n)
                    x2n = wk_.tile([D, BLK], BF16, tag=f"x2{n}")
                    nc.vector.tensor_tensor(x2n[:], x2ps[n][:], rbb[:], AL.mult)
                    x2.append(x2n)
                st["x2"] = x2

            def phase4(st, b):
                r0 = (b % nblk) * BLK
                x2 = st["x2"]
                g_ps = psp.tile([NB, BLK], F32, tag="sv", bufs=1)
                for n in range(NB):
                    mm(g_ps[:], W["gwT"][:, n * NB:(n + 1) * NB], x2[n][:],
                       start=(n == 0), stop=(n == 2))
                e_sb = wk_.tile([NB, BLK], BF16, tag="esb", bufs=1)
                nc.scalar.activation(e_sb[:], g_ps[:], AF.Exp,
                                     bias=W["gateb"][:NB, 0:1])
                zb_ps = psp.tile([D, BLK], F32, tag="ps")
                mm(zb_ps[:], W["ones3"][:NB, :], e_sb[:])
                mns = []
                for n in range(NB):
                    eb_ps = psp.tile([D, BLK], F32, tag="ps")
                    mm(eb_ps[:], W["esel"][:NB, n * D:(n + 1) * D], e_sb[:])
                    mn = wk_.tile([D, BLK], BF16, tag=f"mn{n}", bufs=1)
                    nc.vector.tensor_tensor(mn[:], x2[n][:], eb_ps[:], AL.mult)
                    mns.append(mn)
                acc = wk_.tile([D, BLK], BF16, tag="macc", bufs=1)
                nc.vector.tensor_tensor(acc[:], mns[0][:], mns[1][:], AL.add)
                acc2 = wk_.tile([D, BLK], BF16, tag="macc2", bufs=1)
                nc.vector.tensor_tensor(acc2[:], acc[:], mns[2][:], AL.add)
                rz = wk_.tile([D, BLK], BF16, tag="rz", bufs=1)
                nc.vector.reciprocal(rz[:], zb_ps[:])
                fused = wk_.tile([D, BLK], BF16, tag="fused", bufs=1)
                nc.vector.tensor_tensor(fused[:], acc2[:], rz[:], AL.mult)
                nc.gpsimd.dma_start(out[:, r0:r0 + BLK], fused[:])

            # software-pipelined emission, 6 deep:
            #   t: load(t) | p1(t-1) | p2(t-2) | p3a(t-3) | p3b(t-4) | p4(t-5)
            # gelu clusters (phase3a) alternate head/tail of the tick so the
            # ACT gelu table set loads once per 2 ticks on average.
            total = nblk * repeat
            bstate = {}
            for t in range(total + 6):
                if 0 <= t - 3 < total:
                    phase3a(bstate[t - 3])
                if 0 <= t - 2 < total:
                    phase2(bstate[t - 2])
                if 0 <= t - 5 < total:
                    phase4(bstate.pop(t - 5), t - 5)
                if 0 <= t - 1 < total:
                    phase1(bstate[t - 1])
                if 0 <= t - 4 < total:
                    phase3b(bstate[t - 4])
                if t < total:
                    bstate[t] = phase0(t)
    _fix_wait_overflow(nc)
    return nc


def prep_x(inputs, Bc=None):
    """Host-side: cast to bf16 and pre-transpose into feature-major HBM
    layouts."""
    xsp = np.ascontiguousarray(inputs["x_spatial"]).astype(NPBF)
    B = xsp.shape[0]
    xspT = np.ascontiguousarray(xsp.reshape(B, 10, D).transpose(2, 1, 0))
    xgT = np.ascontiguousarray(inputs["x_gradient"].T.astype(NPBF))
    xfT = np.ascontiguousarray(inputs["x_frequency"].T.astype(NPBF))
    return {"xspT": xspT, "xgT": xgT, "xfT": xfT}


def kernel(**inputs):
    _patch_tile_drain()
    B = inputs["x_spatial"].shape[0]
    Bc = B // NCORES
    w = prep_weights(inputs)
    xb = prep_x(inputs)
    nc = build_program(Bc)
    in_maps = []
    for c in range(NCORES):
        m = dict(w)
        m["xspT"] = np.ascontiguousarray(xb["xspT"][:, :, c * Bc:(c + 1) * Bc])
        m["xgT"] = np.ascontiguousarray(xb["xgT"][:, c * Bc:(c + 1) * Bc])
        m["xfT"] = np.ascontiguousarray(xb["xfT"][:, c * Bc:(c + 1) * Bc])
        in_maps.append(m)
    res = run_bass_kernel_spmd(nc, in_maps, list(range(NCORES)))
    outs = [res.results[c]["outT"] for c in range(NCORES)]
    full = np.concatenate([o.T for o in outs], axis=0)
    return np.ascontiguousarray(full.astype(np.float32))


# revision 20
# speedup vs baseline: 1.9439x; 1.3823x over previous
"""Trainium2 Bass kernel for nn_CMAF (cross-modal attention fusion block).

Layout: feature-major activations on-chip — every tile is
[128 features (partitions) x 1024 samples (free)], so all matmuls are
weight-stationary bf16 with the batch as the moving free dimension.
Inputs are pre-transposed host-side into feature-major HBM layouts, so
device DMA is fully contiguous (no DMA-transpose).

Engine-balance design (ACT/DVE were the baseline bottleneck):
 - LN stats (sum of squares) for all 3 branches land in ONE [3,1024]
   PSUM tile; Ln+Exp (rsqrt) run once per LN stage on that compact tile
   instead of per-branch full tiles; per-branch ones-matmuls broadcast
   the result back to 128 partitions (PE pump is cheap).
 - Residual adds (u = o + P, x2p = f + x1) are folded into the PE as
   identity-matrix accumulation matmuls, killing 1x-rate STT DVE ops.
 - Wo@v1 is folded host-side into Wov = (C Wo) Wv and accumulated into
   the same PSUM as Wo@tp, killing the tpv add.
 - The 2-way attention softmax collapses to division by (1+exp(-d/sqrt(dh)))
   done as a single DVE tensor_tensor divide straight from PSUM.
 - Gelu ACT ops are clustered at alternating head/tail of the pipeline
   tick so the ACT table set (gelu vs natural_log_exp) switches once per
   block on average instead of twice.

Data parallel over 8 NeuronCores: 8192 samples each.
"""

import numpy as np
import ml_dtypes

import concourse.bass as bass
import concourse.mybir as mybir
from concourse.tile import TileContext
from concourse.vector_clock import ScopedClock
from concourse.bass_utils import run_bass_kernel_spmd

F32 = mybir.dt.float32
BF16 = mybir.dt.bfloat16
AL = mybir.AluOpType
AF = mybir.ActivationFunctionType
NPBF = ml_dtypes.bfloat16

D = 128
SP = 1280
FFN = 256
NB = 3
DH = 32
KV_IDX = ((1, 2), (0, 2), (0, 1))
NCORES = 8
BLK = 1024
MMN = 512
EPS = 1e-5
ISQ = float(1.0 / np.sqrt(DH))

# tuning flags
IDENT_FOLD = True      # residual adds via identity matmuls on PE

# filled by build_program: [(phase_label, [instruction names]), ...]
PHASE_MARKS = []


def _patch_tile_drain():
    """walrus here rejects >4 sem waits on one instruction; Tile's tail
    drain carries one wait per logical proc.  Re-emit them as standalone
    wait_ge instructions ahead of the drain."""
    TC = TileContext
    if getattr(TC, "_drain_patched", False):
        return

    def patched(self, tick_clock, wait_clock):
        nop_inst = self.nc.sync.nop()
        wait_clock.add_sem_waits(
            nop_inst.ins, ScopedClock({None: tick_clock.global_clock})
        )
        d = nop_inst.ins
        si = d.sync_info
        waits = list(si.on_wait) if si is not None else []
        if len(waits) > 4:
            si.on_wait = []
            d.sync_info = si
            name2sem = {s.name: s for s in self.sems.allocated().values()}
            for w in waits:
                sem = name2sem.get(w.ant_name)
                if sem is None:
                    raise RuntimeError(f"drain patch: unknown sem {w.ant_name}")
                self.nc.sync.wait_ge(sem, w.wait_value)
        self.nc.sync.drain()
        self.nc.all_engine_barrier()
        popped = self.nc._tile_sem_poison_stack.pop()
        assert popped is self._sem_poison
        self.nc.clear_and_free_semaphores(list(self.sems.allocated().values()))
        self.nc.all_engine_barrier()

    TC._drain_and_barrier = patched
    TC._drain_patched = True


def _fix_wait_overflow(nc):
    """walrus enforces per-opcode caps on sync-wait commands attached to
    one instruction (DmaTransposeAnt: 1, others: ~4).  Move the excess
    onto same-engine NOPs inserted immediately before the instruction."""
    LIMITS = {}
    DEFAULT_LIM = 1
    for fn in nc.m.functions:
        for bb in fn.blocks:
            insts = list(bb.instructions)
            out = []
            changed = False
            for inst in insts:
                si = getattr(inst, "sync_info", None)
                w = list(si.on_wait) if si is not None and si.on_wait else []
                lim = LIMITS.get(type(inst).__name__, DEFAULT_LIM)
                if len(w) > lim:
                    excess = w[lim:]
                    keep = w[:lim]
                    eng = nc.engines[inst.engine]
                    nops = []
                    for i in range(0, len(excess), 1):
                        chunk = excess[i:i + 1]
                        nop_bi = eng.nop()
                        nop_inst = nop_bi.ins
                        cb = nc.cur_bb.bb
                        cb.instructions = [x for x in cb.instructions
                                           if x.name != nop_inst.name]
                        import bass_rust
                        nop_inst.sync_info = bass_rust.SyncInfo(
                            on_wait=chunk, on_update=[])
                        nops.append(nop_inst)
                    si.on_wait = keep
                    inst.sync_info = si
                    out.extend(nops)
                    changed = True
                out.append(inst)
            if changed:
                bb.instructions = out


def prep_weights(inp):
    """Host-side prep of all weights into SBUF layouts. bf16 for matmul
    operands, fp32 for per-partition bias vectors."""
    f64 = np.float64
    C = np.eye(D, dtype=f64) - 1.0 / D

    def bf(a):
        return np.ascontiguousarray(a.astype(np.float32)).astype(NPBF)

    def f32(a):
        return np.ascontiguousarray(a, dtype=np.float32)

    w = {}
    wsp = C @ inp["proj_w_spatial"].astype(f64)            # [128,1280]
    w["wspT"] = bf(np.transpose(wsp.reshape(D, 10, D), (2, 1, 0)).reshape(D, 10 * D))
    wgf = np.stack([C @ inp["proj_w_gf"][i].astype(f64) for i in range(2)])
    w["wgfT"] = bf(np.transpose(wgf, (2, 0, 1)).reshape(D, 2 * D))
    w["bc"] = f32(C @ inp["proj_b"].astype(f64).T)         # [128,3]
    w["emb"] = f32(inp["mod_emb"].T)

    ipw = inp["in_proj_w"].astype(f64)                     # [3, 384, 128]
    wq, wk, wv = ipw[:, :D], ipw[:, D:2 * D], ipw[:, 2 * D:]
    w["wqT"] = bf(np.transpose(wq, (2, 0, 1)).reshape(D, NB * D))
    w["wkT"] = bf(np.transpose(wk, (2, 0, 1)).reshape(D, NB * D))
    w["wvT"] = bf(np.transpose(wv, (2, 0, 1)).reshape(D, NB * D))
    ow = np.stack([C @ inp["out_proj_w"][n].astype(f64) for n in range(NB)])
    # 0.5x fold: attention prob a = (1+tanh(d/(2 sqrt(dh))))/2, the 1/2 is
    # folded here so tp = (tanh+1)*dv feeds Wo directly
    w["owT"] = bf(0.5 * np.transpose(ow, (2, 0, 1)).reshape(D, NB * D))
    # Wov[n] = (C @ Wo[n]) @ Wv[n] : folds the v1 path into one matmul
    wov = np.stack([ow[n] @ wv[n] for n in range(NB)])
    w["wovT"] = bf(np.transpose(wov, (2, 0, 1)).reshape(D, NB * D))
    ob2 = np.stack([
        C @ inp["out_proj_b"][n].astype(f64)
        - inp["mod_emb"][n].astype(f64).mean()
        for n in range(NB)])
    w["ob2"] = f32(ob2.T)

    w1 = inp["ffn_w1"].astype(f64)                         # [3, 256, 128]
    w["w1T"] = bf(np.transpose(w1, (2, 0, 1)).reshape(D, NB * FFN))
    w["b1"] = f32(inp["ffn_b1"].reshape(NB * 2, D).T)      # [128, 6]
    w2 = np.stack([C @ inp["ffn_w2"][n].astype(f64) for n in range(NB)])
    w2c = w2.reshape(NB, D, 2, D)                          # [n, j, c, p]
    w["w2T"] = bf(np.transpose(w2c, (3, 0, 2, 1)).reshape(D, NB * 2 * D))
    b2c = np.stack([C @ inp["ffn_b2"][n].astype(f64) for n in range(NB)])
    w["b2c"] = f32(b2c.T)

    gw = inp["gate_w"].astype(f64).reshape(NB, NB, D)      # [j, n, p]
    w["gwT"] = bf(np.transpose(gw, (2, 1, 0)).reshape(D, NB * NB))
    w["gateb"] = f32(inp["gate_b"].reshape(NB, 1))

    w["onesT"] = bf(np.full((D, D), 1.0 / D))
    svsel = np.zeros((D, NB * NB), dtype=np.float32)
    for n in range(NB):
        svsel[:, NB * n + n] = 1.0 / D
    w["svsel"] = bf(svsel)
    hs = np.zeros((D, D), dtype=np.float32)
    for h in range(4):
        hs[h * DH:(h + 1) * DH, h * DH:(h + 1) * DH] = 1.0
    w["hsel"] = bf(hs)
    w["ones3"] = bf(np.ones((NB, D)))
    esel = np.zeros((NB, NB * D), dtype=np.float32)
    for n in range(NB):
        esel[n, n * D:(n + 1) * D] = 1.0
    w["esel"] = bf(esel)
    w["ident"] = bf(np.eye(D))
    w["epsv"] = np.full((D, 1), EPS, dtype=np.float32)
    w["zerov"] = np.zeros((D, 1), dtype=np.float32)

    assert np.allclose(inp["proj_ln_g"], 1) and np.allclose(inp["proj_ln_b"], 0)
    assert np.allclose(inp["attn_ln_g"], 1) and np.allclose(inp["attn_ln_b"], 0)
    assert np.allclose(inp["ffn_ln_g"], 1) and np.allclose(inp["ffn_ln_b"], 0)
    assert np.allclose(inp["in_proj_b"], 0)
    return w


WEIGHT_SPECS = {
    "wspT": ((D, 10 * D), BF16), "wgfT": ((D, 2 * D), BF16),
    "bc": ((D, NB), F32), "emb": ((D, NB), F32),
    "wqT": ((D, NB * D), BF16), "wkT": ((D, NB * D), BF16),
    "wvT": ((D, NB * D), BF16), "owT": ((D, NB * D), BF16),
    "wovT": ((D, NB * D), BF16),
    "ob2": ((D, NB), F32),
    "w1T": ((D, NB * FFN), BF16), "b1": ((D, NB * 2), F32),
    "w2T": ((D, NB * 2 * D), BF16), "b2c": ((D, NB), F32),
    "gwT": ((D, NB * NB), BF16), "gateb": ((NB, 1), F32),
    "onesT": ((D, D), BF16), "hsel": ((D, D), BF16),
    "svsel": ((D, NB * NB), BF16),
    "ones3": ((NB, D), BF16), "esel": ((NB, NB * D), BF16),
    "ident": ((D, D), BF16),
    "epsv": ((D, 1), F32), "zerov": ((D, 1), F32),
}


def build_program(Bc, repeat=1):
    nc = bass.Bass()
    # pre-transposed feature-major inputs in HBM
    xsp = nc.dram_tensor("xspT", [D, 10, Bc], BF16, kind="ExternalInput")
    xg = nc.dram_tensor("xgT", [D, Bc], BF16, kind="ExternalInput")
    xf = nc.dram_tensor("xfT", [D, Bc], BF16, kind="ExternalInput")
    wd = {k: nc.dram_tensor(k, list(s[0]), s[1], kind="ExternalInput")
          for k, s in WEIGHT_SPECS.items()}
    out = nc.dram_tensor("outT", [D, Bc], BF16, kind="ExternalOutput")

    nblk = Bc // BLK
    assert Bc % BLK == 0

    with TileContext(nc) as tc, nc.allow_low_precision(reason="bf16 kernel"):
        with (
            tc.tile_pool(name="wp", bufs=1) as wp,
            tc.tile_pool(name="xin", bufs=2) as xin,
            tc.tile_pool(name="work", bufs=2) as wk_,
            tc.tile_pool(name="ps", bufs=4, space="PSUM") as psp,
        ):
            W = {}
            for k, s in WEIGHT_SPECS.items():
                W[k] = wp.tile(list(s[0]), s[1], tag=k, name=k)
                nc.gpsimd.dma_start(W[k][:], wd[k][:])
            ident = W["ident"]

            def mm(out_ap, lhsT, rhs, start=True, stop=True):
                for h in range(BLK // MMN):
                    nc.tensor.matmul(out_ap[:, h * MMN:(h + 1) * MMN], lhsT,
                                     rhs[:, h * MMN:(h + 1) * MMN],
                                     start=start, stop=stop)

            def phase0(b):
                r0 = (b % nblk) * BLK
                st = {}
                xspT_all = xin.tile([D, 10 * BLK], BF16, tag="xspT")
                nc.sync.dma_start(
                    xspT_all[:].rearrange("p (c n) -> p c n", c=10),
                    xsp[:, :, r0:r0 + BLK])
                st["xspT"] = xspT_all
                st["xgT"] = xin.tile([D, BLK], BF16, tag="xgT", name="xgT")
                nc.sync.dma_start(st["xgT"][:], xg[:, r0:r0 + BLK])
                st["xfT"] = xin.tile([D, BLK], BF16, tag="xfT", name="xfT")
                nc.sync.dma_start(st["xfT"][:], xf[:, r0:r0 + BLK])
                return st

            def compact_rsqrt(sv_ps, tag):
                """sv_ps: [NB, BLK] PSUM AP of per-branch mean-squares.
                Returns [NB, BLK] bf16 SBUF tile of rsqrt(v+eps)."""
                lnv = wk_.tile([NB, BLK], F32, tag="lnv", bufs=1)
                nc.scalar.activation(lnv[:], sv_ps,
                                     AF.Ln, bias=W["epsv"][:NB, 0:1])
                rbc = wk_.tile([NB, BLK], BF16, tag="rbc", bufs=2)
                nc.scalar.activation(rbc[:], lnv[:], AF.Exp, scale=-0.5,
                                     bias=W["zerov"][:NB, 0:1])
                return rbc

            def bcast(rbc, n):
                """broadcast row n of [NB, BLK] tile to [128, BLK] PSUM."""
                rbb = psp.tile([D, BLK], F32, tag="ps")
                mm(rbb[:], W["esel"][:NB, n * D:(n + 1) * D], rbc[:])
                return rbb

            def phase1(st):
                z_ps = []
                zs = psp.tile([D, BLK], F32, tag="ps")
                for c in range(10):
                    mm(zs[:], W["wspT"][:, c * D:(c + 1) * D],
                       st["xspT"][:, c * BLK:(c + 1) * BLK],
                       start=(c == 0), stop=(c == 9))
                z_ps.append(zs)
                for i, key in ((0, "xgT"), (1, "xfT")):
                    zt = psp.tile([D, BLK], F32, tag="ps")
                    mm(zt[:], W["wgfT"][:, i * D:(i + 1) * D], st[key][:])
                    z_ps.append(zt)
                zsb, sq = [], []
                for n in range(NB):
                    z_sb = wk_.tile([D, BLK], BF16, tag=f"zsb{n}", bufs=1)
                    nc.scalar.activation(z_sb[:], z_ps[n][:], AF.Identity,
                                         bias=W["bc"][:, n:n + 1])
                    zsb.append(z_sb)
                    s_ = wk_.tile([D, BLK], BF16, tag="sq1", bufs=1)
                    nc.vector.tensor_tensor(s_[:], z_sb[:], z_sb[:], AL.mult)
                    sq.append(s_)
                svt = psp.tile([D, BLK], F32, tag="ps")
                sv = svt[0:NB, :]
                for n in range(NB):
                    mm(sv, W["svsel"][:, NB * n:NB * (n + 1)], sq[n][:],
                       start=(n == 0), stop=(n == NB - 1))
                rbc = compact_rsqrt(sv, "1")
                P = []
                for n in range(NB):
                    rbb = bcast(rbc, n)
                    p_ = wk_.tile([D, BLK], BF16, tag=f"P{n}")
                    nc.vector.tensor_tensor(p_[:], zsb[n][:], rbb[:], AL.mult)
                    nc.vector.tensor_scalar_add(p_[:], p_[:], W["emb"][:, n:n + 1])
                    P.append(p_)
                st["P"] = P
                dP = []
                for n in range(NB):
                    s0, s1 = KV_IDX[n]
                    dp = wk_.tile([D, BLK], BF16, tag=f"dP{n}", bufs=2)
                    nc.vector.tensor_tensor(dp[:], P[s0][:], P[s1][:], AL.subtract)
                    dP.append(dp)
                st["dP"] = dP

            def phase2(st):
                P = st["P"]
                dP = st["dP"]
                q_ps, dk_ps, dv_ps, d_ps, o_ps = {}, {}, {}, {}, {}
                q_sb, t0, th, tp, us, sqs = {}, {}, {}, {}, [], []
                for n in range(NB):
                    q_ps[n] = psp.tile([D, BLK], F32, tag="ps", name=f"qps{n}")
                    mm(q_ps[n][:], W["wqT"][:, n * D:(n + 1) * D], P[n][:])
                    dk_ps[n] = psp.tile([D, BLK], F32, tag="ps", name=f"dkps{n}")
                    mm(dk_ps[n][:], W["wkT"][:, n * D:(n + 1) * D], dP[n][:])
                    q_sb[n] = wk_.tile([D, BLK], BF16, tag="qsb", bufs=2, name=f"qsb{n}")
                    nc.scalar.activation(q_sb[n][:], q_ps[n][:], AF.Copy)
                    t0[n] = wk_.tile([D, BLK], BF16, tag="t0", bufs=2, name=f"t0{n}")
                    nc.vector.tensor_tensor(t0[n][:], q_sb[n][:], dk_ps[n][:],
                                            AL.mult)
                for n in range(NB):
                    dv_ps[n] = psp.tile([D, BLK], F32, tag="ps", name=f"dvps{n}")
                    mm(dv_ps[n][:], W["wvT"][:, n * D:(n + 1) * D], dP[n][:])
                    d_ps[n] = psp.tile([D, BLK], F32, tag="ps", name=f"dps{n}")
                    mm(d_ps[n][:], W["hsel"][:], t0[n][:])
                    th[n] = wk_.tile([D, BLK], BF16, tag="th", bufs=2, name=f"th{n}")
                    nc.scalar.activation(th[n][:], d_ps[n][:], AF.Tanh,
                                         bias=W["zerov"][:, 0:1],
                                         scale=0.5 * ISQ)
                    tp[n] = wk_.tile([D, BLK], BF16, tag="tp", bufs=2, name=f"tp{n}")
                    nc.vector.scalar_tensor_tensor(
                        tp[n][:], th[n][:], 1.0, dv_ps[n][:], AL.add, AL.mult)
                for n in range(NB):
                    s0, s1 = KV_IDX[n]
                    o_ps[n] = psp.tile([D, BLK], F32, tag="ps", name=f"ops{n}")
                    mm(o_ps[n][:], W["owT"][:, n * D:(n + 1) * D], tp[n][:],
                       start=True, stop=False)
                    mm(o_ps[n][:], W["wovT"][:, n * D:(n + 1) * D], P[s1][:],
                       start=False, stop=not IDENT_FOLD)
                    if IDENT_FOLD:
                        mm(o_ps[n][:], ident[:], P[n][:], start=False, stop=True)
                        u = wk_.tile([D, BLK], BF16, tag=f"u{n}", bufs=1)
                        nc.scalar.activation(u[:], o_ps[n][:], AF.Identity,
                                             bias=W["ob2"][:, n:n + 1])
                    else:
                        u = wk_.tile([D, BLK], BF16, tag=f"u{n}", bufs=1)
                        nc.vector.scalar_tensor_tensor(
                            u[:], o_ps[n][:], W["ob2"][:, n:n + 1], P[n][:],
                            AL.add, AL.add)
                    us.append(u)
                    s_ = wk_.tile([D, BLK], BF16, tag="sq2", bufs=1)
                    nc.vector.tensor_tensor(s_[:], u[:], u[:], AL.mult)
                    sqs.append(s_)
                svt = psp.tile([D, BLK], F32, tag="ps")
                sv = svt[0:NB, :]
                for n in range(NB):
                    mm(sv, W["svsel"][:, NB * n:NB * (n + 1)], sqs[n][:],
                       start=(n == 0), stop=(n == NB - 1))
                rbc = compact_rsqrt(sv, "2")
                x1 = []
                for n in range(NB):
                    rbb = bcast(rbc, n)
                    # x1 lives 3 ticks: made in p2(t-2), read by p3a(t-3)
                    # and p3b(t-4)
                    x1n = wk_.tile([D, BLK], BF16, tag=f"x1{n}", bufs=3)
                    nc.vector.tensor_tensor(x1n[:], us[n][:], rbb[:], AL.mult)
                    x1.append(x1n)
                st["x1"] = x1

            def phase3a(st):
                """FFN first half: W1 matmuls + gelu cluster."""
                x1 = st["x1"]
                hs_all = []
                for n in range(NB):
                    h_sb = []
                    for c in range(2):
                        h_ps = psp.tile([D, BLK], F32, tag="ps")
                        mm(h_ps[:],
                           W["w1T"][:, n * FFN + c * D: n * FFN + (c + 1) * D],
                           x1[n][:])
                        hs_ = wk_.tile([D, BLK], BF16, tag=f"hsb{n}_{c}", bufs=2)
                        nc.scalar.activation(hs_[:], h_ps[:], AF.Gelu,
                                             bias=W["b1"][:, 2 * n + c: 2 * n + c + 1])
                        h_sb.append(hs_)
                    hs_all.append(h_sb)
                st["hs"] = hs_all

            def phase3b(st):
                x1 = st["x1"]
                x2ps, sqs = [], []
                for n in range(NB):
                    h_sb = st["hs"][n]
                    f_ps = psp.tile([D, BLK], F32, tag="ps")
                    for c in range(2):
                        mm(f_ps[:], W["w2T"][:, (2 * n + c) * D:(2 * n + c + 1) * D],
                           h_sb[c][:], start=(c == 0),
                           stop=(c == 1 and not IDENT_FOLD))
                    if IDENT_FOLD:
                        mm(f_ps[:], ident[:], x1[n][:], start=False, stop=True)
                        x2p = wk_.tile([D, BLK], BF16, tag=f"x2p{n}", bufs=1)
                        nc.scalar.activation(x2p[:], f_ps[:], AF.Identity,
                                             bias=W["b2c"][:, n:n + 1])
                    else:
                        x2p = wk_.tile([D, BLK], BF16, tag=f"x2p{n}", bufs=1)
                        nc.vector.scalar_tensor_tensor(
                            x2p[:], f_ps[:], W["b2c"][:, n:n + 1], x1[n][:],
                            AL.add, AL.add)
                    x2ps.append(x2p)
                    s_ = wk_.tile([D, BLK], BF16, tag="sq3", bufs=1)
                    nc.vector.tensor_tensor(s_[:], x2p[:], x2p[:], AL.mult)
                    sqs.append(s_)
                svt = psp.tile([D, BLK], F32, tag="ps")
                sv = svt[0:NB, :]
                for n in range(NB):
                    mm(sv, W["svsel"][:, NB * n:NB * (n + 1)], sqs[n][:],
                       start=(n == 0), stop=(n == NB - 1))
                rbc = compact_rsqrt(sv, "3")
                x2 = []
                for n in range(NB):
                    rbb = bcast(rbc, n)
                    x2n = wk_.tile([D, BLK], BF16, tag=f"x2{n}")
                    nc.vector.tensor_tensor(x2n[:], x2ps[n][:], rbb[:], AL.mult)
                    x2.append(x2n)
                st["x2"] = x2

            def phase4(st, b):
                r0 = (b % nblk) * BLK
                x2 = st["x2"]
                g_pst = psp.tile([D, BLK], F32, tag="ps")
                g_ps = g_pst[0:NB, :]
                for n in range(NB):
                    mm(g_ps, W["gwT"][:, n * NB:(n + 1) * NB], x2[n][:],
                       start=(n == 0), stop=(n == 2))
                e_sb = wk_.tile([NB, BLK], BF16, tag="esb", bufs=1)
                nc.scalar.activation(e_sb[:], g_ps, AF.Exp,
                                     bias=W["gateb"][:NB, 0:1])
                zb_ps = psp.tile([D, BLK], F32, tag="ps")
                mm(zb_ps[:], W["ones3"][:NB, :], e_sb[:])
                mns = []
                for n in range(NB):
                    eb_ps = psp.tile([D, BLK], F32, tag="ps")
                    mm(eb_ps[:], W["esel"][:NB, n * D:(n + 1) * D], e_sb[:])
                    mn = wk_.tile([D, BLK], BF16, tag=f"mn{n}", bufs=1)
                    nc.vector.tensor_tensor(mn[:], x2[n][:], eb_ps[:], AL.mult)
                    mns.append(mn)
                acc = wk_.tile([D, BLK], BF16, tag="macc", bufs=1)
                nc.vector.tensor_tensor(acc[:], mns[0][:], mns[1][:], AL.add)
                acc2 = wk_.tile([D, BLK], BF16, tag="macc2", bufs=1)
                nc.vector.tensor_tensor(acc2[:], acc[:], mns[2][:], AL.add)
                rz = wk_.tile([D, BLK], BF16, tag="rz", bufs=1)
                nc.vector.reciprocal(rz[:], zb_ps[:])
                fused = wk_.tile([D, BLK], BF16, tag="fused", bufs=1)
                nc.vector.tensor_tensor(fused[:], acc2[:], rz[:], AL.mult)
                nc.gpsimd.dma_start(out[:, r0:r0 + BLK], fused[:])

            # software-pipelined emission, 6 deep:
            #   t: load(t) | p1(t-1) | p2(t-2) | p3a(t-3) | p3b(t-4) | p4(t-5)
            # gelu clusters (phase3a) alternate head/tail of the tick so the
            # ACT gelu table set loads once per 2 ticks on average.
            total = nblk * repeat
            bstate = {}
            marks = PHASE_MARKS
            marks.clear()

            def _run(label, fn, *a):
                i0 = len(nc.cur_bb.bb.instructions)
                fn(*a)
                i1 = len(nc.cur_bb.bb.instructions)
                marks.append((label, [x.name for x in
                                      nc.cur_bb.bb.instructions[i0:i1]]))

            for t in range(total + 6):
                if 0 <= t - 3 < total:
                    _run("p3a", phase3a, bstate[t - 3])
                if 0 <= t - 2 < total:
                    _run("p2", phase2, bstate[t - 2])
                if 0 <= t - 5 < total:
                    _run("p4", phase4, bstate.pop(t - 5), t - 5)
                if 0 <= t - 1 < total:
                    _run("p1", phase1, bstate[t - 1])
                if 0 <= t - 4 < total:
                    _run("p3b", phase3b, bstate[t - 4])
                if t < total:
                    bstate[t] = None
                    _run("p0", lambda tt=t: bstate.__setitem__(tt, phase0(tt)))
    _fix_wait_overflow(nc)
    return nc


def prep_x(inputs, Bc=None):
    """Host-side: cast to bf16 and pre-transpose into feature-major HBM
    layouts."""
    xsp = np.ascontiguousarray(inputs["x_spatial"]).astype(NPBF)
    B = xsp.shape[0]
    xspT = np.ascontiguousarray(xsp.reshape(B, 10, D).transpose(2, 1, 0))
    xgT = np.ascontiguousarray(inputs["x_gradient"].T.astype(NPBF))
    xfT = np.ascontiguousarray(inputs["x_frequency"].T.astype(NPBF))
    return {"xspT": xspT, "xgT": xgT, "xfT": xfT}


def kernel(**inputs):
    _patch_tile_drain()
    B = inputs["x_spatial"].shape[0]
    Bc = B // NCORES
    w = prep_weights(inputs)
    xb = prep_x(inputs)
    nc = build_program(Bc)
    in_maps = []
    for c in range(NCORES):
        m = dict(w)
        m["xspT"] = np.ascontiguousarray(xb["xspT"][:, :, c * Bc:(c + 1) * Bc])
        m["xgT"] = np.ascontiguousarray(xb["xgT"][:, c * Bc:(c + 1) * Bc])
        m["xfT"] = np.ascontiguousarray(xb["xfT"][:, c * Bc:(c + 1) * Bc])
        in_maps.append(m)
    res = run_bass_kernel_spmd(nc, in_maps, list(range(NCORES)))
    outs = [res.results[c]["outT"] for c in range(NCORES)]
    full = np.concatenate([o.T for o in outs], axis=0)
    return np.ascontiguousarray(full.astype(np.float32))


# revision 27
# speedup vs baseline: 2.2537x; 1.1594x over previous
"""Trainium2 Bass kernel for nn_CMAF (cross-modal attention fusion block).

Layout: feature-major activations on-chip — every tile is
[128 features (partitions) x 1024 samples (free)], so all matmuls are
weight-stationary bf16 with the batch as the moving free dimension.
Inputs are pre-transposed host-side into feature-major HBM layouts, so
device DMA is fully contiguous (no DMA-transpose).

Engine-balance design (ACT/DVE were the baseline bottleneck):
 - LN stats (sum of squares) for all 3 branches land in ONE [3,1024]
   PSUM tile; Ln+Exp (rsqrt) run once per LN stage on that compact tile
   instead of per-branch full tiles; per-branch ones-matmuls broadcast
   the result back to 128 partitions (PE pump is cheap).
 - Residual adds (u = o + P, x2p = f + x1) are folded into the PE as
   identity-matrix accumulation matmuls, killing 1x-rate STT DVE ops.
 - Wo@v1 is folded host-side into Wov = (C Wo) Wv and accumulated into
   the same PSUM as Wo@tp, killing the tpv add.
 - The 2-way attention softmax collapses to division by (1+exp(-d/sqrt(dh)))
   done as a single DVE tensor_tensor divide straight from PSUM.
 - Gelu ACT ops are clustered at alternating head/tail of the pipeline
   tick so the ACT table set (gelu vs natural_log_exp) switches once per
   block on average instead of twice.

Data parallel over 8 NeuronCores: 8192 samples each.
"""

import numpy as np
import ml_dtypes

import concourse.bass as bass
import concourse.mybir as mybir
from concourse.tile import TileContext
from concourse.vector_clock import ScopedClock
from concourse.bass_utils import run_bass_kernel_spmd

F32 = mybir.dt.float32
BF16 = mybir.dt.bfloat16
AL = mybir.AluOpType
AF = mybir.ActivationFunctionType
NPBF = ml_dtypes.bfloat16

D = 128
SP = 1280
FFN = 256
NB = 3
DH = 32
KV_IDX = ((1, 2), (0, 2), (0, 1))
NCORES = 8
BLK = 1024
MMN = 512
EPS = 1e-5
ISQ = float(1.0 / np.sqrt(DH))

# tuning flags
IDENT_FOLD = True      # residual adds via identity matmuls on PE

# filled by build_program: [(phase_label, [instruction names]), ...]
PHASE_MARKS = []


def _patch_tile_drain():
    """walrus here rejects >4 sem waits on one instruction; Tile's tail
    drain carries one wait per logical proc.  Re-emit them as standalone
    wait_ge instructions ahead of the drain."""
    TC = TileContext
    if getattr(TC, "_drain_patched", False):
        return

    def patched(self, tick_clock, wait_clock):
        nop_inst = self.nc.sync.nop()
        wait_clock.add_sem_waits(
            nop_inst.ins, ScopedClock({None: tick_clock.global_clock})
        )
        d = nop_inst.ins
        si = d.sync_info
        waits = list(si.on_wait) if si is not None else []
        if len(waits) > 4:
            si.on_wait = []
            d.sync_info = si
            name2sem = {s.name: s for s in self.sems.allocated().values()}
            for w in waits:
                sem = name2sem.get(w.ant_name)
                if sem is None:
                    raise RuntimeError(f"drain patch: unknown sem {w.ant_name}")
                self.nc.sync.wait_ge(sem, w.wait_value)
        self.nc.sync.drain()
        self.nc.all_engine_barrier()
        popped = self.nc._tile_sem_poison_stack.pop()
        assert popped is self._sem_poison
        self.nc.clear_and_free_semaphores(list(self.sems.allocated().values()))
        self.nc.all_engine_barrier()

    TC._drain_and_barrier = patched
    TC._drain_patched = True


def _fix_wait_overflow(nc):
    """walrus enforces per-opcode caps on sync-wait commands attached to
    one instruction (DmaTransposeAnt: 1, others: ~4).  Move the excess
    onto same-engine NOPs inserted immediately before the instruction."""
    LIMITS = {}
    DEFAULT_LIM = 1
    for fn in nc.m.functions:
        for bb in fn.blocks:
            insts = list(bb.instructions)
            out = []
            changed = False
            for inst in insts:
                si = getattr(inst, "sync_info", None)
                w = list(si.on_wait) if si is not None and si.on_wait else []
                lim = LIMITS.get(type(inst).__name__, DEFAULT_LIM)
                if len(w) > lim:
                    excess = w[lim:]
                    keep = w[:lim]
                    eng = nc.engines[inst.engine]
                    nops = []
                    for i in range(0, len(excess), 1):
                        chunk = excess[i:i + 1]
                        nop_bi = eng.nop()
                        nop_inst = nop_bi.ins
                        cb = nc.cur_bb.bb
                        cb.instructions = [x for x in cb.instructions
                                           if x.name != nop_inst.name]
                        import bass_rust
                        nop_inst.sync_info = bass_rust.SyncInfo(
                            on_wait=chunk, on_update=[])
                        nops.append(nop_inst)
                    si.on_wait = keep
                    inst.sync_info = si
                    out.extend(nops)
                    changed = True
                out.append(inst)
            if changed:
                bb.instructions = out


def prep_weights(inp):
    """Host-side prep of all weights into SBUF layouts. bf16 for matmul
    operands, fp32 for per-partition bias vectors."""
    f64 = np.float64
    C = np.eye(D, dtype=f64) - 1.0 / D

    def bf(a):
        return np.ascontiguousarray(a.astype(np.float32)).astype(NPBF)

    def f32(a):
        return np.ascontiguousarray(a, dtype=np.float32)

    w = {}
    wsp = C @ inp["proj_w_spatial"].astype(f64)            # [128,1280]
    w["wspT"] = bf(np.transpose(wsp.reshape(D, 10, D), (2, 1, 0)).reshape(D, 10 * D))
    wgf = np.stack([C @ inp["proj_w_gf"][i].astype(f64) for i in range(2)])
    w["wgfT"] = bf(np.transpose(wgf, (2, 0, 1)).reshape(D, 2 * D))
    w["bc"] = f32(C @ inp["proj_b"].astype(f64).T)         # [128,3]
    w["emb"] = f32(inp["mod_emb"].T)

    ipw = inp["in_proj_w"].astype(f64)                     # [3, 384, 128]
    wq, wk, wv = ipw[:, :D], ipw[:, D:2 * D], ipw[:, 2 * D:]
    w["wqT"] = bf(np.transpose(wq, (2, 0, 1)).reshape(D, NB * D))
    w["wkT"] = bf(np.transpose(wk, (2, 0, 1)).reshape(D, NB * D))
    w["wvT"] = bf(np.transpose(wv, (2, 0, 1)).reshape(D, NB * D))
    ow = np.stack([C @ inp["out_proj_w"][n].astype(f64) for n in range(NB)])
    # 0.5x fold: attention prob a = (1+tanh(d/(2 sqrt(dh))))/2, the 1/2 is
    # folded here so tp = (tanh+1)*dv feeds Wo directly
    w["owT"] = bf(0.5 * np.transpose(ow, (2, 0, 1)).reshape(D, NB * D))
    # Wov[n] = (C @ Wo[n]) @ Wv[n] : folds the v1 path into one matmul
    wov = np.stack([ow[n] @ wv[n] for n in range(NB)])
    w["wovT"] = bf(np.transpose(wov, (2, 0, 1)).reshape(D, NB * D))
    ob2 = np.stack([
        C @ inp["out_proj_b"][n].astype(f64)
        - inp["mod_emb"][n].astype(f64).mean()
        for n in range(NB)])
    w["ob2"] = f32(ob2.T)

    w1 = inp["ffn_w1"].astype(f64)                         # [3, 256, 128]
    w["w1T"] = bf(np.transpose(w1, (2, 0, 1)).reshape(D, NB * FFN))
    w["b1"] = f32(inp["ffn_b1"].reshape(NB * 2, D).T)      # [128, 6]
    w2 = np.stack([C @ inp["ffn_w2"][n].astype(f64) for n in range(NB)])
    w2c = w2.reshape(NB, D, 2, D)                          # [n, j, c, p]
    w["w2T"] = bf(np.transpose(w2c, (3, 0, 2, 1)).reshape(D, NB * 2 * D))
    b2c = np.stack([C @ inp["ffn_b2"][n].astype(f64) for n in range(NB)])
    w["b2c"] = f32(b2c.T)

    gw = inp["gate_w"].astype(f64).reshape(NB, NB, D)      # [j, n, p]
    w["gwT"] = bf(np.transpose(gw, (2, 1, 0)).reshape(D, NB * NB))
    w["gateb"] = f32(inp["gate_b"].reshape(NB, 1))

    w["onesT"] = bf(np.full((D, D), 1.0 / D))
    svsel = np.zeros((D, NB * NB), dtype=np.float32)
    for n in range(NB):
        svsel[:, NB * n + n] = 1.0 / D
    w["svsel"] = bf(svsel)
    hs = np.zeros((D, D), dtype=np.float32)
    for h in range(4):
        hs[h * DH:(h + 1) * DH, h * DH:(h + 1) * DH] = 1.0
    w["hsel"] = bf(hs)
    w["ones3"] = bf(np.ones((NB, D)))
    esel = np.zeros((NB, NB * D), dtype=np.float32)
    for n in range(NB):
        esel[n, n * D:(n + 1) * D] = 1.0
    w["esel"] = bf(esel)
    w["ident"] = bf(np.eye(D))
    w["epsv"] = np.full((D, 1), EPS, dtype=np.float32)
    w["zerov"] = np.zeros((D, 1), dtype=np.float32)

    assert np.allclose(inp["proj_ln_g"], 1) and np.allclose(inp["proj_ln_b"], 0)
    assert np.allclose(inp["attn_ln_g"], 1) and np.allclose(inp["attn_ln_b"], 0)
    assert np.allclose(inp["ffn_ln_g"], 1) and np.allclose(inp["ffn_ln_b"], 0)
    assert np.allclose(inp["in_proj_b"], 0)
    return w


WEIGHT_SPECS = {
    "wspT": ((D, 10 * D), BF16), "wgfT": ((D, 2 * D), BF16),
    "bc": ((D, NB), F32), "emb": ((D, NB), F32),
    "wqT": ((D, NB * D), BF16), "wkT": ((D, NB * D), BF16),
    "wvT": ((D, NB * D), BF16), "owT": ((D, NB * D), BF16),
    "wovT": ((D, NB * D), BF16),
    "ob2": ((D, NB), F32),
    "w1T": ((D, NB * FFN), BF16), "b1": ((D, NB * 2), F32),
    "w2T": ((D, NB * 2 * D), BF16), "b2c": ((D, NB), F32),
    "gwT": ((D, NB * NB), BF16), "gateb": ((NB, 1), F32),
    "onesT": ((D, D), BF16), "hsel": ((D, D), BF16),
    "svsel": ((D, NB * NB), BF16),
    "ones3": ((NB, D), BF16), "esel": ((NB, NB * D), BF16),
    "ident": ((D, D), BF16),
    "epsv": ((D, 1), F32), "zerov": ((D, 1), F32),
}


def build_program(Bc, repeat=1):
    nc = bass.Bass()
    # pre-transposed feature-major inputs in HBM
    xsp = nc.dram_tensor("xspT", [D, 10, Bc], BF16, kind="ExternalInput")
    xg = nc.dram_tensor("xgT", [D, Bc], BF16, kind="ExternalInput")
    xf = nc.dram_tensor("xfT", [D, Bc], BF16, kind="ExternalInput")
    wd = {k: nc.dram_tensor(k, list(s[0]), s[1], kind="ExternalInput")
          for k, s in WEIGHT_SPECS.items()}
    out = nc.dram_tensor("outT", [D, Bc], BF16, kind="ExternalOutput")

    nblk = Bc // BLK
    assert Bc % BLK == 0

    with TileContext(nc) as tc, nc.allow_low_precision(reason="bf16 kernel"):
        with (
            tc.tile_pool(name="wp", bufs=1) as wp,
            tc.tile_pool(name="xin", bufs=2) as xin,
            tc.tile_pool(name="work", bufs=2) as wk_,
            tc.tile_pool(name="ps", bufs=8, space="PSUM") as psp,
        ):
            W = {}
            for k, s in WEIGHT_SPECS.items():
                W[k] = wp.tile(list(s[0]), s[1], tag=k, name=k)
                nc.gpsimd.dma_start(W[k][:], wd[k][:])
            ident = W["ident"]

            def mm(out_ap, lhsT, rhs, start=True, stop=True):
                for h in range(BLK // MMN):
                    nc.tensor.matmul(out_ap[:, h * MMN:(h + 1) * MMN], lhsT,
                                     rhs[:, h * MMN:(h + 1) * MMN],
                                     start=start, stop=stop)

            def phase0(b):
                r0 = (b % nblk) * BLK
                st = {}
                for half, nmh in ((0, "xspA"), (1, "xspB")):
                    xt = xin.tile([D, 5 * BLK], BF16, tag="xspT", bufs=3,
                                  name=nmh)
                    nc.sync.dma_start(
                        xt[:].rearrange("p (c n) -> p c n", c=5),
                        xsp[:, 5 * half:5 * (half + 1), r0:r0 + BLK])
                    st[nmh] = xt
                st["xgT"] = xin.tile([D, BLK], BF16, tag="xgT", name="xgT")
                nc.sync.dma_start(st["xgT"][:], xg[:, r0:r0 + BLK])
                st["xfT"] = xin.tile([D, BLK], BF16, tag="xfT", name="xfT")
                nc.sync.dma_start(st["xfT"][:], xf[:, r0:r0 + BLK])
                return st

            NH = BLK // MMN   # psum halves per logical [D, BLK] tile

            def hmm(name, pairs, parts=D):
                """Accumulating matmul into NH independent psum half tiles.
                pairs: [(lhsT_ap, full-width SBUF rhs AP)] accumulated.
                Emits half 0 fully, then half 1 (so half-0 consumers can
                start while half 1 accumulates).  Returns list of halves."""
                halves = []
                for h in range(NH):
                    pt = psp.tile([D, MMN], F32, tag="ps", name=f"{name}_{h}")
                    ap = pt[0:parts, :] if parts != D else pt[:]
                    for i, (lhsT, rhs) in enumerate(pairs):
                        nc.tensor.matmul(ap, lhsT,
                                         rhs[:, h * MMN:(h + 1) * MMN],
                                         start=(i == 0),
                                         stop=(i == len(pairs) - 1))
                    halves.append((pt, ap))
                return halves

            def for_halves(halves, sbuf_op):
                """sbuf_op(h, lo, hi, psum_ap) for each half."""
                for h, (pt, ap) in enumerate(halves):
                    sbuf_op(h, h * MMN, (h + 1) * MMN, ap)

            def compact_rsqrt(sv_halves, tag):
                """sv halves: [NB, MMN] PSUM of per-branch mean squares.
                Returns [NB, BLK] bf16 SBUF rsqrt(v+eps)."""
                lnv = wk_.tile([NB, BLK], F32, tag="lnv", bufs=1)
                for_halves(sv_halves, lambda h, lo, hi, ap:
                           nc.scalar.activation(lnv[:, lo:hi], ap, AF.Ln,
                                                bias=W["epsv"][:NB, 0:1]))
                rbc = wk_.tile([NB, BLK], BF16, tag=f"rbc{tag}", bufs=2)
                nc.scalar.activation(rbc[:], lnv[:], AF.Exp, scale=-0.5,
                                     bias=W["zerov"][:NB, 0:1])
                return rbc

            def bcast_apply(rbc, n, src_sb, out_sb, name):
                """out = src * broadcast(row n of rbc), per half."""
                sel = W["esel"][:NB, n * D:(n + 1) * D]
                for h in range(NH):
                    lo, hi = h * MMN, (h + 1) * MMN
                    rbb = psp.tile([D, MMN], F32, tag="ps",
                                   name=f"{name}_{h}")
                    nc.tensor.matmul(rbb[:], sel, rbc[:, lo:hi],
                                     start=True, stop=True)
                    nc.vector.tensor_tensor(out_sb[:, lo:hi], src_sb[:, lo:hi],
                                            rbb[:], AL.mult)

            def p1a(st):
                """projections + drains + squares + LN1 stats/rsqrt"""
                zh = []
                zh.append(hmm("zsp", [
                    (W["wspT"][:, c * D:(c + 1) * D],
                     st["xspA" if c < 5 else "xspB"][:, (c % 5) * BLK:
                                                     (c % 5 + 1) * BLK])
                    for c in range(10)]))
                zh.append(hmm("zg", [(W["wgfT"][:, 0:D], st["xgT"][:])]))
                zh.append(hmm("zf", [(W["wgfT"][:, D:2 * D], st["xfT"][:])]))
                zsb, sq = [], []
                for n in range(NB):
                    z_sb = wk_.tile([D, BLK], BF16, tag=f"zsb{n}", bufs=2)
                    for_halves(zh[n], lambda h, lo, hi, ap:
                               nc.scalar.activation(z_sb[:, lo:hi], ap,
                                                    AF.Identity,
                                                    bias=W["bc"][:, n:n + 1]))
                    zsb.append(z_sb)
                    s_ = wk_.tile([D, BLK], BF16, tag="sq1", bufs=1)
                    nc.vector.tensor_tensor(s_[:], z_sb[:], z_sb[:], AL.mult)
                    sq.append(s_)
                sv = hmm("sv1", [(W["svsel"][:, NB * n:NB * (n + 1)],
                                  sq[n][:]) for n in range(NB)], parts=NB)
                st["zsb"] = zsb
                st["rbc1"] = compact_rsqrt(sv, "1")

            def p1b(st):
                """LN1 broadcast + apply + emb + dP"""
                zsb = st.pop("zsb")
                rbc = st.pop("rbc1")
                P = []
                for n in range(NB):
                    p_ = wk_.tile([D, BLK], BF16, tag=f"P{n}")
                    bcast_apply(rbc, n, zsb[n], p_, f"rbb1_{n}")
                    nc.vector.tensor_scalar_add(p_[:], p_[:],
                                                W["emb"][:, n:n + 1])
                    P.append(p_)
                st["P"] = P
                dP = []
                for n in range(NB):
                    s0, s1 = KV_IDX[n]
                    dp = wk_.tile([D, BLK], BF16, tag=f"dP{n}", bufs=2)
                    nc.vector.tensor_tensor(dp[:], P[s0][:], P[s1][:],
                                            AL.subtract)
                    dP.append(dp)
                st["dP"] = dP

            def p2a(st):
                """q/dk matmuls, q drain, score product"""
                P, dP = st["P"], st["dP"]
                t0 = {}
                for n in range(NB):
                    qh = hmm(f"q{n}", [(W["wqT"][:, n * D:(n + 1) * D],
                                        P[n][:])])
                    dkh = hmm(f"dk{n}", [(W["wkT"][:, n * D:(n + 1) * D],
                                          dP[n][:])])
                    q_sb = wk_.tile([D, BLK], BF16, tag="qsb", bufs=2,
                                    name=f"qsb{n}")
                    for_halves(qh, lambda h, lo, hi, ap:
                               nc.scalar.activation(q_sb[:, lo:hi], ap,
                                                    AF.Copy))
                    t0[n] = wk_.tile([D, BLK], BF16, tag="t0", bufs=2,
                                     name=f"t0{n}")
                    for_halves(dkh, lambda h, lo, hi, ap:
                               nc.vector.tensor_tensor(t0[n][:, lo:hi],
                                                       q_sb[:, lo:hi], ap,
                                                       AL.mult))
                st["t0"] = t0

            def p2b(st):
                """dv/score-bcast matmuls, tanh, fused (tanh+1)*dv"""
                dP = st["dP"]
                t0 = st.pop("t0")
                tp = {}
                for n in range(NB):
                    dvh = hmm(f"dv{n}", [(W["wvT"][:, n * D:(n + 1) * D],
                                          dP[n][:])])
                    dh = hmm(f"d{n}", [(W["hsel"][:], t0[n][:])])
                    th = wk_.tile([D, BLK], BF16, tag="th", bufs=2,
                                  name=f"th{n}")
                    for_halves(dh, lambda h, lo, hi, ap:
                               nc.scalar.activation(th[:, lo:hi], ap, AF.Tanh,
                                                    bias=W["zerov"][:, 0:1],
                                                    scale=0.5 * ISQ))
                    tp[n] = wk_.tile([D, BLK], BF16, tag="tp", bufs=2,
                                     name=f"tp{n}")
                    for_halves(dvh, lambda h, lo, hi, ap:
                               nc.vector.scalar_tensor_tensor(
                                   tp[n][:, lo:hi], th[:, lo:hi], 1.0, ap,
                                   AL.add, AL.mult))
                st["tp"] = tp

            def p2c(st):
                """attention out + residual (PE-folded) + LN2 stats/rsqrt"""
                P = st["P"]
                tp = st.pop("tp")
                us, sqs = [], []
                for n in range(NB):
                    s0, s1 = KV_IDX[n]
                    oh = hmm(f"o{n}",
                             [(W["owT"][:, n * D:(n + 1) * D], tp[n][:]),
                              (W["wovT"][:, n * D:(n + 1) * D], P[s1][:]),
                              (ident[:], P[n][:])])
                    u = wk_.tile([D, BLK], BF16, tag=f"u{n}", bufs=2)
                    for_halves(oh, lambda h, lo, hi, ap:
                               nc.scalar.activation(u[:, lo:hi], ap,
                                                    AF.Identity,
                                                    bias=W["ob2"][:, n:n + 1]))
                    us.append(u)
                    s_ = wk_.tile([D, BLK], BF16, tag="sq2", bufs=1)
                    nc.vector.tensor_tensor(s_[:], u[:], u[:], AL.mult)
                    sqs.append(s_)
                sv = hmm("sv2", [(W["svsel"][:, NB * n:NB * (n + 1)],
                                  sqs[n][:]) for n in range(NB)], parts=NB)
                st["us"] = us
                st["rbc2"] = compact_rsqrt(sv, "2")

            def p2d(st):
                """LN2 broadcast + apply"""
                us = st.pop("us")
                rbc = st.pop("rbc2")
                x1 = []
                for n in range(NB):
                    # x1 lives 3 ticks: made here (t-2), read by p3a(t-3)
                    # and p3b1(t-4)
                    x1n = wk_.tile([D, BLK], BF16, tag=f"x1{n}", bufs=3)
                    bcast_apply(rbc, n, us[n], x1n, f"rbb2_{n}")
                    x1.append(x1n)
                st["x1"] = x1

            def phase3a(st):
                """FFN first half: W1 matmuls + gelu cluster."""
                x1 = st["x1"]
                hs_all = []
                for n in range(NB):
                    h_sb = []
                    for c in range(2):
                        hh = hmm(f"h{n}_{c}",
                                 [(W["w1T"][:, n * FFN + c * D:
                                            n * FFN + (c + 1) * D],
                                   x1[n][:])])
                        hs_ = wk_.tile([D, BLK], BF16, tag=f"hsb{n}_{c}",
                                       bufs=1)
                        for_halves(hh, lambda h, lo, hi, ap:
                                   nc.scalar.activation(
                                       hs_[:, lo:hi], ap, AF.Gelu,
                                       bias=W["b1"][:, 2 * n + c:
                                                    2 * n + c + 1]))
                        h_sb.append(hs_)
                    hs_all.append(h_sb)
                st["hs"] = hs_all

            def p3b1(st):
                """FFN second half + residual + LN3 stats/rsqrt"""
                x1 = st["x1"]
                x2ps, sqs = [], []
                for n in range(NB):
                    h_sb = st["hs"][n]
                    fh = hmm(f"f{n}",
                             [(W["w2T"][:, (2 * n) * D:(2 * n + 1) * D],
                               h_sb[0][:]),
                              (W["w2T"][:, (2 * n + 1) * D:(2 * n + 2) * D],
                               h_sb[1][:]),
                              (ident[:], x1[n][:])])
                    x2p = wk_.tile([D, BLK], BF16, tag=f"x2p{n}", bufs=2)
                    for_halves(fh, lambda h, lo, hi, ap:
                               nc.scalar.activation(x2p[:, lo:hi], ap,
                                                    AF.Identity,
                                                    bias=W["b2c"][:, n:n + 1]))
                    x2ps.append(x2p)
                    s_ = wk_.tile([D, BLK], BF16, tag="sq3", bufs=1)
                    nc.vector.tensor_tensor(s_[:], x2p[:], x2p[:], AL.mult)
                    sqs.append(s_)
                sv = hmm("sv3", [(W["svsel"][:, NB * n:NB * (n + 1)],
                                  sqs[n][:]) for n in range(NB)], parts=NB)
                st["x2ps"] = x2ps
                st["rbc3"] = compact_rsqrt(sv, "3")

            def p3b2(st):
                """LN3 broadcast + apply"""
                x2ps = st.pop("x2ps")
                rbc = st.pop("rbc3")
                st.pop("hs")
                x2 = []
                for n in range(NB):
                    x2n = wk_.tile([D, BLK], BF16, tag=f"x2{n}")
                    bcast_apply(rbc, n, x2ps[n], x2n, f"rbb3_{n}")
                    x2.append(x2n)
                st["x2"] = x2

            def phase4(st, b):
                r0 = (b % nblk) * BLK
                x2 = st["x2"]
                gh = hmm("g", [(W["gwT"][:, n * NB:(n + 1) * NB], x2[n][:])
                               for n in range(NB)], parts=NB)
                e_sb = wk_.tile([NB, BLK], BF16, tag="esb", bufs=1)
                for_halves(gh, lambda h, lo, hi, ap:
                           nc.scalar.activation(e_sb[:, lo:hi], ap, AF.Exp,
                                                bias=W["gateb"][:NB, 0:1]))
                zbh = hmm("zb", [(W["ones3"][:NB, :], e_sb[:])])
                rz = wk_.tile([D, BLK], BF16, tag="rz", bufs=1)
                for_halves(zbh, lambda h, lo, hi, ap:
                           nc.vector.reciprocal(rz[:, lo:hi], ap))
                mns = []
                for n in range(NB):
                    ebh = hmm(f"eb{n}", [(W["esel"][:NB, n * D:(n + 1) * D],
                                          e_sb[:])])
                    mn = wk_.tile([D, BLK], BF16, tag=f"mn{n}", bufs=1)
                    for_halves(ebh, lambda h, lo, hi, ap:
                               nc.vector.tensor_tensor(mn[:, lo:hi],
                                                       x2[n][:, lo:hi], ap,
                                                       AL.mult))
                    mns.append(mn)
                acc = wk_.tile([D, BLK], BF16, tag="macc", bufs=1)
                nc.vector.tensor_tensor(acc[:], mns[0][:], mns[1][:], AL.add)
                acc2 = wk_.tile([D, BLK], BF16, tag="macc2", bufs=1)
                nc.vector.tensor_tensor(acc2[:], acc[:], mns[2][:], AL.add)
                fused = wk_.tile([D, BLK], BF16, tag="fused", bufs=1)
                nc.vector.tensor_tensor(fused[:], acc2[:], rz[:], AL.mult)
                nc.gpsimd.dma_start(out[:, r0:r0 + BLK], fused[:])

            # sub-phase interleaved emission, 6 blocks in flight; psum is
            # 8 independent [D, MMN] half-tile slots.  ACT table sets:
            # gelu+tanh (gelu_and_others) at the tick head, all Ln/Exp
            # users after -> 2 table switches per tick.
            total = nblk * repeat
            bstate = {}
            marks = PHASE_MARKS
            marks.clear()

            def _run(label, fn, *a):
                i0 = len(nc.cur_bb.bb.instructions)
                fn(*a)
                i1 = len(nc.cur_bb.bb.instructions)
                marks.append((label, [x.name for x in
                                      nc.cur_bb.bb.instructions[i0:i1]]))

            for t in range(total + 8):
                if 0 <= t - 5 < total:
                    _run("p3a", phase3a, bstate[t - 5])
                if 0 <= t - 3 < total:
                    _run("p2a", p2a, bstate[t - 3])
                    _run("p2b", p2b, bstate[t - 3])
                if 0 <= t - 1 < total:
                    _run("p1a", p1a, bstate[t - 1])
                if 0 <= t - 3 < total:
                    _run("p2c", p2c, bstate[t - 3])
                if 0 <= t - 5 < total:
                    _run("p3b1", p3b1, bstate[t - 5])
                if 0 <= t - 2 < total:
                    _run("p1b", p1b, bstate[t - 2])
                if 0 <= t - 4 < total:
                    _run("p2d", p2d, bstate[t - 4])
                if 0 <= t - 6 < total:
                    _run("p3b2", p3b2, bstate[t - 6])
                if 0 <= t - 7 < total:
                    _run("p4", phase4, bstate[t - 7], t - 7)
                    bstate.pop(t - 7)
                if t < total:
                    bstate[t] = None
                    _run("p0", lambda tt=t: bstate.__setitem__(tt, phase0(tt)))
    _fix_wait_overflow(nc)
    return nc


def prep_x(inputs, Bc=None):
    """Host-side: cast to bf16 and pre-transpose into feature-major HBM
    layouts."""
    xsp = np.ascontiguousarray(inputs["x_spatial"]).astype(NPBF)
    B = xsp.shape[0]
    xspT = np.ascontiguousarray(xsp.reshape(B, 10, D).transpose(2, 1, 0))
    xgT = np.ascontiguousarray(inputs["x_gradient"].T.astype(NPBF))
    xfT = np.ascontiguousarray(inputs["x_frequency"].T.astype(NPBF))
    return {"xspT": xspT, "xgT": xgT, "xfT": xfT}


def kernel(**inputs):
    _patch_tile_drain()
    B = inputs["x_spatial"].shape[0]
    Bc = B // NCORES
    w = prep_weights(inputs)
    xb = prep_x(inputs)
    nc = build_program(Bc)
    in_maps = []
    for c in range(NCORES):
        m = dict(w)
        m["xspT"] = np.ascontiguousarray(xb["xspT"][:, :, c * Bc:(c + 1) * Bc])
        m["xgT"] = np.ascontiguousarray(xb["xgT"][:, c * Bc:(c + 1) * Bc])
        m["xfT"] = np.ascontiguousarray(xb["xfT"][:, c * Bc:(c + 1) * Bc])
        in_maps.append(m)
    res = run_bass_kernel_spmd(nc, in_maps, list(range(NCORES)))
    outs = [res.results[c]["outT"] for c in range(NCORES)]
    full = np.concatenate([o.T for o in outs], axis=0)
    return np.ascontiguousarray(full.astype(np.float32))


# revision 29
# speedup vs baseline: 5.5912x; 2.4809x over previous
"""Trainium2 Bass kernel for nn_CMAF (cross-modal attention fusion block).

Layout: feature-major activations on-chip — every tile is
[128 features (partitions) x 1024 samples (free)], so all matmuls are
weight-stationary bf16 with the batch as the moving free dimension.
Inputs are pre-transposed host-side into feature-major HBM layouts, so
device DMA is fully contiguous (no DMA-transpose).

Engine-balance design (ACT/DVE were the baseline bottleneck):
 - LN stats (sum of squares) for all 3 branches land in ONE [3,1024]
   PSUM tile; Ln+Exp (rsqrt) run once per LN stage on that compact tile
   instead of per-branch full tiles; per-branch ones-matmuls broadcast
   the result back to 128 partitions (PE pump is cheap).
 - Residual adds (u = o + P, x2p = f + x1) are folded into the PE as
   identity-matrix accumulation matmuls, killing 1x-rate STT DVE ops.
 - Wo@v1 is folded host-side into Wov = (C Wo) Wv and accumulated into
   the same PSUM as Wo@tp, killing the tpv add.
 - The 2-way attention softmax collapses to division by (1+exp(-d/sqrt(dh)))
   done as a single DVE tensor_tensor divide straight from PSUM.
 - Gelu ACT ops are clustered at alternating head/tail of the pipeline
   tick so the ACT table set (gelu vs natural_log_exp) switches once per
   block on average instead of twice.

Data parallel over 8 NeuronCores: 8192 samples each.
"""

import numpy as np
import ml_dtypes

import concourse.bass as bass
import concourse.mybir as mybir
from concourse.tile import TileContext
from concourse.vector_clock import ScopedClock
from concourse.bass_utils import run_bass_kernel_spmd

F32 = mybir.dt.float32
BF16 = mybir.dt.bfloat16
AL = mybir.AluOpType
AF = mybir.ActivationFunctionType
NPBF = ml_dtypes.bfloat16

D = 128
SP = 1280
FFN = 256
NB = 3
DH = 32
KV_IDX = ((1, 2), (0, 2), (0, 1))
NCORES = 8
BLK = 1024
MMN = 512
EPS = 1e-5
ISQ = float(1.0 / np.sqrt(DH))

# tuning flags
IDENT_FOLD = True      # residual adds via identity matmuls on PE

# filled by build_program: [(phase_label, [instruction names]), ...]
PHASE_MARKS = []


def _patch_tile_drain():
    """walrus here rejects >4 sem waits on one instruction; Tile's tail
    drain carries one wait per logical proc.  Re-emit them as standalone
    wait_ge instructions ahead of the drain."""
    TC = TileContext
    if getattr(TC, "_drain_patched", False):
        return

    def patched(self, tick_clock, wait_clock):
        nop_inst = self.nc.sync.nop()
        wait_clock.add_sem_waits(
            nop_inst.ins, ScopedClock({None: tick_clock.global_clock})
        )
        d = nop_inst.ins
        si = d.sync_info
        waits = list(si.on_wait) if si is not None else []
        if len(waits) > 4:
            si.on_wait = []
            d.sync_info = si
            name2sem = {s.name: s for s in self.sems.allocated().values()}
            for w in waits:
                sem = name2sem.get(w.ant_name)
                if sem is None:
                    raise RuntimeError(f"drain patch: unknown sem {w.ant_name}")
                self.nc.sync.wait_ge(sem, w.wait_value)
        self.nc.sync.drain()
        self.nc.all_engine_barrier()
        popped = self.nc._tile_sem_poison_stack.pop()
        assert popped is self._sem_poison
        self.nc.clear_and_free_semaphores(list(self.sems.allocated().values()))
        self.nc.all_engine_barrier()

    TC._drain_and_barrier = patched
    TC._drain_patched = True


def _fix_wait_overflow(nc):
    """walrus enforces per-opcode caps on sync-wait commands attached to
    one instruction (DmaTransposeAnt: 1, others: ~4).  Move the excess
    onto same-engine NOPs inserted immediately before the instruction."""
    LIMITS = {}
    DEFAULT_LIM = 1
    for fn in nc.m.functions:
        for bb in fn.blocks:
            insts = list(bb.instructions)
            out = []
            changed = False
            for inst in insts:
                si = getattr(inst, "sync_info", None)
                w = list(si.on_wait) if si is not None and si.on_wait else []
                lim = LIMITS.get(type(inst).__name__, DEFAULT_LIM)
                if len(w) > lim:
                    excess = w[lim:]
                    keep = w[:lim]
                    eng = nc.engines[inst.engine]
                    nops = []
                    for i in range(0, len(excess), 1):
                        chunk = excess[i:i + 1]
                        nop_bi = eng.nop()
                        nop_inst = nop_bi.ins
                        cb = nc.cur_bb.bb
                        cb.instructions = [x for x in cb.instructions
                                           if x.name != nop_inst.name]
                        import bass_rust
                        nop_inst.sync_info = bass_rust.SyncInfo(
                            on_wait=chunk, on_update=[])
                        nops.append(nop_inst)
                    si.on_wait = keep
                    inst.sync_info = si
                    out.extend(nops)
                    changed = True
                out.append(inst)
            if changed:
                bb.instructions = out


def prep_weights(inp):
    """Host-side prep of all weights into SBUF layouts. bf16 for matmul
    operands, fp32 for per-partition bias vectors."""
    f64 = np.float64
    C = np.eye(D, dtype=f64) - 1.0 / D

    def bf(a):
        return np.ascontiguousarray(a.astype(np.float32)).astype(NPBF)

    def f32(a):
        return np.ascontiguousarray(a, dtype=np.float32)

    w = {}
    wsp = C @ inp["proj_w_spatial"].astype(f64)            # [128,1280]
    w["wspT"] = bf(np.transpose(wsp.reshape(D, 10, D), (2, 1, 0)).reshape(D, 10 * D))
    wgf = np.stack([C @ inp["proj_w_gf"][i].astype(f64) for i in range(2)])
    w["wgfT"] = bf(np.transpose(wgf, (2, 0, 1)).reshape(D, 2 * D))
    w["bc"] = f32(C @ inp["proj_b"].astype(f64).T)         # [128,3]
    w["emb"] = f32(inp["mod_emb"].T)

    ipw = inp["in_proj_w"].astype(f64)                     # [3, 384, 128]
    wq, wk, wv = ipw[:, :D], ipw[:, D:2 * D], ipw[:, 2 * D:]
    w["wqT"] = bf(np.transpose(wq, (2, 0, 1)).reshape(D, NB * D))
    w["wkT"] = bf(np.transpose(wk, (2, 0, 1)).reshape(D, NB * D))
    w["wvT"] = bf(np.transpose(wv, (2, 0, 1)).reshape(D, NB * D))
    ow = np.stack([C @ inp["out_proj_w"][n].astype(f64) for n in range(NB)])
    # 0.5x fold: attention prob a = (1+tanh(d/(2 sqrt(dh))))/2, the 1/2 is
    # folded here so tp = (tanh+1)*dv feeds Wo directly
    w["owT"] = bf(0.5 * np.transpose(ow, (2, 0, 1)).reshape(D, NB * D))
    # Wov[n] = (C @ Wo[n]) @ Wv[n] : folds the v1 path into one matmul
    wov = np.stack([ow[n] @ wv[n] for n in range(NB)])
    w["wovT"] = bf(np.transpose(wov, (2, 0, 1)).reshape(D, NB * D))
    ob2 = np.stack([
        C @ inp["out_proj_b"][n].astype(f64)
        - inp["mod_emb"][n].astype(f64).mean()
        for n in range(NB)])
    w["ob2"] = f32(ob2.T)

    w1 = inp["ffn_w1"].astype(f64)                         # [3, 256, 128]
    w["w1T"] = bf(np.transpose(w1, (2, 0, 1)).reshape(D, NB * FFN))
    w["b1"] = f32(inp["ffn_b1"].reshape(NB * 2, D).T)      # [128, 6]
    w2 = np.stack([C @ inp["ffn_w2"][n].astype(f64) for n in range(NB)])
    w2c = w2.reshape(NB, D, 2, D)                          # [n, j, c, p]
    w["w2T"] = bf(np.transpose(w2c, (3, 0, 2, 1)).reshape(D, NB * 2 * D))
    b2c = np.stack([C @ inp["ffn_b2"][n].astype(f64) for n in range(NB)])
    w["b2c"] = f32(b2c.T)

    gw = inp["gate_w"].astype(f64).reshape(NB, NB, D)      # [j, n, p]
    w["gwT"] = bf(np.transpose(gw, (2, 1, 0)).reshape(D, NB * NB))
    w["gateb"] = f32(inp["gate_b"].reshape(NB, 1))

    w["onesT"] = bf(np.full((D, D), 1.0 / D))
    svsel = np.zeros((D, NB * NB), dtype=np.float32)
    for n in range(NB):
        svsel[:, NB * n + n] = 1.0 / D
    w["svsel"] = bf(svsel)
    hs = np.zeros((D, D), dtype=np.float32)
    for h in range(4):
        hs[h * DH:(h + 1) * DH, h * DH:(h + 1) * DH] = 1.0
    w["hsel"] = bf(hs)
    w["ones3"] = bf(np.ones((NB, D)))
    esel = np.zeros((NB, NB * D), dtype=np.float32)
    for n in range(NB):
        esel[n, n * D:(n + 1) * D] = 1.0
    w["esel"] = bf(esel)
    w["ident"] = bf(np.eye(D))
    w["epsv"] = np.full((D, 1), EPS, dtype=np.float32)
    w["zerov"] = np.zeros((D, 1), dtype=np.float32)

    assert np.allclose(inp["proj_ln_g"], 1) and np.allclose(inp["proj_ln_b"], 0)
    assert np.allclose(inp["attn_ln_g"], 1) and np.allclose(inp["attn_ln_b"], 0)
    assert np.allclose(inp["ffn_ln_g"], 1) and np.allclose(inp["ffn_ln_b"], 0)
    assert np.allclose(inp["in_proj_b"], 0)
    return w


WEIGHT_SPECS = {
    "wspT": ((D, 10 * D), BF16), "wgfT": ((D, 2 * D), BF16),
    "bc": ((D, NB), F32), "emb": ((D, NB), F32),
    "wqT": ((D, NB * D), BF16), "wkT": ((D, NB * D), BF16),
    "wvT": ((D, NB * D), BF16), "owT": ((D, NB * D), BF16),
    "wovT": ((D, NB * D), BF16),
    "ob2": ((D, NB), F32),
    "w1T": ((D, NB * FFN), BF16), "b1": ((D, NB * 2), F32),
    "w2T": ((D, NB * 2 * D), BF16), "b2c": ((D, NB), F32),
    "gwT": ((D, NB * NB), BF16), "gateb": ((NB, 1), F32),
    "onesT": ((D, D), BF16), "hsel": ((D, D), BF16),
    "svsel": ((D, NB * NB), BF16),
    "ones3": ((NB, D), BF16), "esel": ((NB, NB * D), BF16),
    "ident": ((D, D), BF16),
    "epsv": ((D, 1), F32), "zerov": ((D, 1), F32),
}


def build_program(Bc, repeat=1):
    nc = bass.Bass()
    # pre-transposed feature-major inputs in HBM
    xsp = nc.dram_tensor("xspT", [D, 10, Bc], BF16, kind="ExternalInput")
    xg = nc.dram_tensor("xgT", [D, Bc], BF16, kind="ExternalInput")
    xf = nc.dram_tensor("xfT", [D, Bc], BF16, kind="ExternalInput")
    wd = {k: nc.dram_tensor(k, list(s[0]), s[1], kind="ExternalInput")
          for k, s in WEIGHT_SPECS.items()}
    out = nc.dram_tensor("outT", [D, Bc], BF16, kind="ExternalOutput")

    nblk = Bc // BLK
    assert Bc % BLK == 0

    with TileContext(nc) as tc, nc.allow_low_precision(reason="bf16 kernel"):
        with (
            tc.tile_pool(name="wp", bufs=1) as wp,
            tc.tile_pool(name="xin", bufs=2) as xin,
            tc.tile_pool(name="work", bufs=2) as wk_,
            tc.tile_pool(name="ps", bufs=8, space="PSUM") as psp,
        ):
            W = {}
            for k, s in WEIGHT_SPECS.items():
                W[k] = wp.tile(list(s[0]), s[1], tag=k, name=k)
                nc.gpsimd.dma_start(W[k][:], wd[k][:])
            ident = W["ident"]

            def mm(out_ap, lhsT, rhs, start=True, stop=True):
                for h in range(BLK // MMN):
                    nc.tensor.matmul(out_ap[:, h * MMN:(h + 1) * MMN], lhsT,
                                     rhs[:, h * MMN:(h + 1) * MMN],
                                     start=start, stop=stop)

            def phase0(b):
                r0 = (b % nblk) * BLK
                st = {}
                for half, nmh in ((0, "xspA"), (1, "xspB")):
                    xt = xin.tile([D, 5 * BLK], BF16, tag="xspT", bufs=3,
                                  name=nmh)
                    nc.sync.dma_start(
                        xt[:].rearrange("p (c n) -> p c n", c=5),
                        xsp[:, 5 * half:5 * (half + 1), r0:r0 + BLK])
                    st[nmh] = xt
                st["xgT"] = xin.tile([D, BLK], BF16, tag="xgT", name="xgT")
                nc.sync.dma_start(st["xgT"][:], xg[:, r0:r0 + BLK])
                st["xfT"] = xin.tile([D, BLK], BF16, tag="xfT", name="xfT")
                nc.sync.dma_start(st["xfT"][:], xf[:, r0:r0 + BLK])
                return st

            NH = BLK // MMN   # psum halves per logical [D, BLK] tile

            def hmm(name, pairs, parts=D):
                """Accumulating matmul into NH independent psum half tiles.
                pairs: [(lhsT_ap, full-width SBUF rhs AP)] accumulated.
                Emits half 0 fully, then half 1 (so half-0 consumers can
                start while half 1 accumulates).  Returns list of halves."""
                halves = []
                for h in range(NH):
                    pt = psp.tile([D, MMN], F32, tag="ps", name=f"{name}_{h}")
                    ap = pt[0:parts, :] if parts != D else pt[:]
                    for i, (lhsT, rhs) in enumerate(pairs):
                        nc.tensor.matmul(ap, lhsT,
                                         rhs[:, h * MMN:(h + 1) * MMN],
                                         start=(i == 0),
                                         stop=(i == len(pairs) - 1))
                    halves.append((pt, ap))
                return halves

            def for_halves(halves, sbuf_op):
                """sbuf_op(h, lo, hi, psum_ap) for each half."""
                for h, (pt, ap) in enumerate(halves):
                    sbuf_op(h, h * MMN, (h + 1) * MMN, ap)

            def compact_rsqrt(sv_halves, tag):
                """sv halves: [NB, MMN] PSUM of per-branch mean squares.
                Returns [NB, BLK] bf16 SBUF rsqrt(v+eps)."""
                lnv = wk_.tile([NB, BLK], F32, tag="lnv", bufs=1)
                for_halves(sv_halves, lambda h, lo, hi, ap:
                           nc.scalar.activation(lnv[:, lo:hi], ap, AF.Ln,
                                                bias=W["epsv"][:NB, 0:1]))
                rbc = wk_.tile([NB, BLK], BF16, tag=f"rbc{tag}", bufs=2)
                nc.scalar.activation(rbc[:], lnv[:], AF.Exp, scale=-0.5,
                                     bias=W["zerov"][:NB, 0:1])
                return rbc

            def bcast_apply(rbc, n, src_sb, out_sb, name):
                """out = src * broadcast(row n of rbc), per half."""
                sel = W["esel"][:NB, n * D:(n + 1) * D]
                for h in range(NH):
                    lo, hi = h * MMN, (h + 1) * MMN
                    rbb = psp.tile([D, MMN], F32, tag="ps",
                                   name=f"{name}_{h}")
                    nc.tensor.matmul(rbb[:], sel, rbc[:, lo:hi],
                                     start=True, stop=True)
                    nc.vector.tensor_tensor(out_sb[:, lo:hi], src_sb[:, lo:hi],
                                            rbb[:], AL.mult)

            def p1a(st):
                """projections + drains + squares + LN1 stats/rsqrt"""
                zh = []
                zh.append(hmm("zsp", [
                    (W["wspT"][:, c * D:(c + 1) * D],
                     st["xspA" if c < 5 else "xspB"][:, (c % 5) * BLK:
                                                     (c % 5 + 1) * BLK])
                    for c in range(10)]))
                zh.append(hmm("zg", [(W["wgfT"][:, 0:D], st["xgT"][:])]))
                zh.append(hmm("zf", [(W["wgfT"][:, D:2 * D], st["xfT"][:])]))
                zsb, sq = [], []
                for n in range(NB):
                    z_sb = wk_.tile([D, BLK], BF16, tag=f"zsb{n}", bufs=2)
                    for_halves(zh[n], lambda h, lo, hi, ap:
                               nc.scalar.activation(z_sb[:, lo:hi], ap,
                                                    AF.Identity,
                                                    bias=W["bc"][:, n:n + 1]))
                    zsb.append(z_sb)
                    s_ = wk_.tile([D, BLK], BF16, tag="sq1", bufs=1)
                    nc.vector.tensor_tensor(s_[:], z_sb[:], z_sb[:], AL.mult)
                    sq.append(s_)
                sv = hmm("sv1", [(W["svsel"][:, NB * n:NB * (n + 1)],
                                  sq[n][:]) for n in range(NB)], parts=NB)
                st["zsb"] = zsb
                st["rbc1"] = compact_rsqrt(sv, "1")

            def p1b(st):
                """LN1 broadcast + apply + emb + dP"""
                zsb = st.pop("zsb")
                rbc = st.pop("rbc1")
                P = []
                for n in range(NB):
                    p_ = wk_.tile([D, BLK], BF16, tag=f"P{n}")
                    bcast_apply(rbc, n, zsb[n], p_, f"rbb1_{n}")
                    nc.vector.tensor_scalar_add(p_[:], p_[:],
                                                W["emb"][:, n:n + 1])
                    P.append(p_)
                st["P"] = P
                dP = []
                for n in range(NB):
                    s0, s1 = KV_IDX[n]
                    dp = wk_.tile([D, BLK], BF16, tag=f"dP{n}", bufs=2)
                    nc.vector.tensor_tensor(dp[:], P[s0][:], P[s1][:],
                                            AL.subtract)
                    dP.append(dp)
                st["dP"] = dP

            def p2a(st):
                """q/dk matmuls, q drain, score product"""
                P, dP = st["P"], st["dP"]
                t0 = {}
                for n in range(NB):
                    qh = hmm(f"q{n}", [(W["wqT"][:, n * D:(n + 1) * D],
                                        P[n][:])])
                    dkh = hmm(f"dk{n}", [(W["wkT"][:, n * D:(n + 1) * D],
                                          dP[n][:])])
                    q_sb = wk_.tile([D, BLK], BF16, tag="qsb", bufs=2,
                                    name=f"qsb{n}")
                    for_halves(qh, lambda h, lo, hi, ap:
                               nc.scalar.activation(q_sb[:, lo:hi], ap,
                                                    AF.Copy))
                    t0[n] = wk_.tile([D, BLK], BF16, tag="t0", bufs=2,
                                     name=f"t0{n}")
                    for_halves(dkh, lambda h, lo, hi, ap:
                               nc.vector.tensor_tensor(t0[n][:, lo:hi],
                                                       q_sb[:, lo:hi], ap,
                                                       AL.mult))
                st["t0"] = t0

            def p2b(st):
                """dv/score-bcast matmuls, tanh, fused (tanh+1)*dv"""
                dP = st["dP"]
                t0 = st.pop("t0")
                tp = {}
                for n in range(NB):
                    dvh = hmm(f"dv{n}", [(W["wvT"][:, n * D:(n + 1) * D],
                                          dP[n][:])])
                    dh = hmm(f"d{n}", [(W["hsel"][:], t0[n][:])])
                    th = wk_.tile([D, BLK], BF16, tag="th", bufs=2,
                                  name=f"th{n}")
                    for_halves(dh, lambda h, lo, hi, ap:
                               nc.scalar.activation(th[:, lo:hi], ap, AF.Tanh,
                                                    bias=W["zerov"][:, 0:1],
                                                    scale=0.5 * ISQ))
                    tp[n] = wk_.tile([D, BLK], BF16, tag="tp", bufs=2,
                                     name=f"tp{n}")
                    for_halves(dvh, lambda h, lo, hi, ap:
                               nc.vector.scalar_tensor_tensor(
                                   tp[n][:, lo:hi], th[:, lo:hi], 1.0, ap,
                                   AL.add, AL.mult))
                st["tp"] = tp

            def p2c(st):
                """attention out + residual (PE-folded) + LN2 stats/rsqrt"""
                P = st["P"]
                tp = st.pop("tp")
                us, sqs = [], []
                for n in range(NB):
                    s0, s1 = KV_IDX[n]
                    oh = hmm(f"o{n}",
                             [(W["owT"][:, n * D:(n + 1) * D], tp[n][:]),
                              (W["wovT"][:, n * D:(n + 1) * D], P[s1][:]),
                              (ident[:], P[n][:])])
                    u = wk_.tile([D, BLK], BF16, tag=f"u{n}", bufs=2)
                    for_halves(oh, lambda h, lo, hi, ap:
                               nc.scalar.activation(u[:, lo:hi], ap,
                                                    AF.Identity,
                                                    bias=W["ob2"][:, n:n + 1]))
                    us.append(u)
                    s_ = wk_.tile([D, BLK], BF16, tag="sq2", bufs=1)
                    nc.vector.tensor_tensor(s_[:], u[:], u[:], AL.mult)
                    sqs.append(s_)
                sv = hmm("sv2", [(W["svsel"][:, NB * n:NB * (n + 1)],
                                  sqs[n][:]) for n in range(NB)], parts=NB)
                st["us"] = us
                st["rbc2"] = compact_rsqrt(sv, "2")

            def p2d(st):
                """LN2 broadcast + apply"""
                us = st.pop("us")
                rbc = st.pop("rbc2")
                x1 = []
                for n in range(NB):
                    # x1 lives 3 ticks: made here (t-2), read by p3a(t-3)
                    # and p3b1(t-4)
                    x1n = wk_.tile([D, BLK], BF16, tag=f"x1{n}", bufs=3)
                    bcast_apply(rbc, n, us[n], x1n, f"rbb2_{n}")
                    x1.append(x1n)
                st["x1"] = x1

            def phase3a(st):
                """FFN first half: W1 matmuls + gelu cluster."""
                x1 = st["x1"]
                hs_all = []
                for n in range(NB):
                    h_sb = []
                    for c in range(2):
                        hh = hmm(f"h{n}_{c}",
                                 [(W["w1T"][:, n * FFN + c * D:
                                            n * FFN + (c + 1) * D],
                                   x1[n][:])])
                        hs_ = wk_.tile([D, BLK], BF16, tag=f"hsb{n}_{c}",
                                       bufs=1)
                        for_halves(hh, lambda h, lo, hi, ap:
                                   nc.scalar.activation(
                                       hs_[:, lo:hi], ap, AF.Gelu,
                                       bias=W["b1"][:, 2 * n + c:
                                                    2 * n + c + 1]))
                        h_sb.append(hs_)
                    hs_all.append(h_sb)
                st["hs"] = hs_all

            def p3b1(st):
                """FFN second half + residual + LN3 stats/rsqrt"""
                x1 = st["x1"]
                x2ps, sqs = [], []
                for n in range(NB):
                    h_sb = st["hs"][n]
                    fh = hmm(f"f{n}",
                             [(W["w2T"][:, (2 * n) * D:(2 * n + 1) * D],
                               h_sb[0][:]),
                              (W["w2T"][:, (2 * n + 1) * D:(2 * n + 2) * D],
                               h_sb[1][:]),
                              (ident[:], x1[n][:])])
                    x2p = wk_.tile([D, BLK], BF16, tag=f"x2p{n}", bufs=2)
                    for_halves(fh, lambda h, lo, hi, ap:
                               nc.scalar.activation(x2p[:, lo:hi], ap,
                                                    AF.Identity,
                                                    bias=W["b2c"][:, n:n + 1]))
                    x2ps.append(x2p)
                    s_ = wk_.tile([D, BLK], BF16, tag="sq3", bufs=1)
                    nc.vector.tensor_tensor(s_[:], x2p[:], x2p[:], AL.mult)
                    sqs.append(s_)
                sv = hmm("sv3", [(W["svsel"][:, NB * n:NB * (n + 1)],
                                  sqs[n][:]) for n in range(NB)], parts=NB)
                st["x2ps"] = x2ps
                st["rbc3"] = compact_rsqrt(sv, "3")

            def p3b2(st):
                """LN3 broadcast + apply"""
                x2ps = st.pop("x2ps")
                rbc = st.pop("rbc3")
                st.pop("hs")
                x2 = []
                for n in range(NB):
                    x2n = wk_.tile([D, BLK], BF16, tag=f"x2{n}")
                    bcast_apply(rbc, n, x2ps[n], x2n, f"rbb3_{n}")
                    x2.append(x2n)
                st["x2"] = x2

            def phase4(st, b):
                r0 = (b % nblk) * BLK
                x2 = st["x2"]
                gh = hmm("g", [(W["gwT"][:, n * NB:(n + 1) * NB], x2[n][:])
                               for n in range(NB)], parts=NB)
                e_sb = wk_.tile([NB, BLK], BF16, tag="esb", bufs=1)
                for_halves(gh, lambda h, lo, hi, ap:
                           nc.scalar.activation(e_sb[:, lo:hi], ap, AF.Exp,
                                                bias=W["gateb"][:NB, 0:1]))
                zbh = hmm("zb", [(W["ones3"][:NB, :], e_sb[:])])
                rz = wk_.tile([D, BLK], BF16, tag="rz", bufs=1)
                for_halves(zbh, lambda h, lo, hi, ap:
                           nc.vector.reciprocal(rz[:, lo:hi], ap))
                mns = []
                for n in range(NB):
                    ebh = hmm(f"eb{n}", [(W["esel"][:NB, n * D:(n + 1) * D],
                                          e_sb[:])])
                    mn = wk_.tile([D, BLK], BF16, tag=f"mn{n}", bufs=1)
                    for_halves(ebh, lambda h, lo, hi, ap:
                               nc.vector.tensor_tensor(mn[:, lo:hi],
                                                       x2[n][:, lo:hi], ap,
                                                       AL.mult))
                    mns.append(mn)
                acc = wk_.tile([D, BLK], BF16, tag="macc", bufs=1)
                nc.vector.tensor_tensor(acc[:], mns[0][:], mns[1][:], AL.add)
                acc2 = wk_.tile([D, BLK], BF16, tag="macc2", bufs=1)
                nc.vector.tensor_tensor(acc2[:], acc[:], mns[2][:], AL.add)
                fused = wk_.tile([D, BLK], BF16, tag="fused", bufs=1)
                nc.vector.tensor_tensor(fused[:], acc2[:], rz[:], AL.mult)
                nc.gpsimd.dma_start(out[:, r0:r0 + BLK], fused[:])

            # sub-phase interleaved emission, 6 blocks in flight; psum is
            # 8 independent [D, MMN] half-tile slots.  ACT table sets:
            # gelu+tanh (gelu_and_others) at the tick head, all Ln/Exp
            # users after -> 2 table switches per tick.
            total = nblk * repeat
            bstate = {}
            marks = PHASE_MARKS
            marks.clear()

            def _run(label, fn, *a):
                i0 = len(nc.cur_bb.bb.instructions)
                fn(*a)
                i1 = len(nc.cur_bb.bb.instructions)
                marks.append((label, [x.name for x in
                                      nc.cur_bb.bb.instructions[i0:i1]]))

            for t in range(total + 8):
                if 0 <= t - 5 < total:
                    _run("p3a", phase3a, bstate[t - 5])
                if 0 <= t - 3 < total:
                    _run("p2a", p2a, bstate[t - 3])
                    _run("p2b", p2b, bstate[t - 3])
                if 0 <= t - 1 < total:
                    _run("p1a", p1a, bstate[t - 1])
                if 0 <= t - 3 < total:
                    _run("p2c", p2c, bstate[t - 3])
                if 0 <= t - 5 < total:
                    _run("p3b1", p3b1, bstate[t - 5])
                if 0 <= t - 2 < total:
                    _run("p1b", p1b, bstate[t - 2])
                if 0 <= t - 4 < total:
                    _run("p2d", p2d, bstate[t - 4])
                if 0 <= t - 6 < total:
                    _run("p3b2", p3b2, bstate[t - 6])
                if 0 <= t - 7 < total:
                    _run("p4", phase4, bstate[t - 7], t - 7)
                    bstate.pop(t - 7)
                if t < total:
                    bstate[t] = None
                    _run("p0", lambda tt=t: bstate.__setitem__(tt, phase0(tt)))
    _fix_wait_overflow(nc)
    return nc


def prep_x(inputs, Bc=None):
    """Host-side: cast to bf16 and pre-transpose into feature-major HBM
    layouts."""
    xsp = np.ascontiguousarray(inputs["x_spatial"]).astype(NPBF)
    B = xsp.shape[0]
    xspT = np.ascontiguousarray(xsp.reshape(B, 10, D).transpose(2, 1, 0))
    xgT = np.ascontiguousarray(inputs["x_gradient"].T.astype(NPBF))
    xfT = np.ascontiguousarray(inputs["x_frequency"].T.astype(NPBF))
    return {"xspT": xspT, "xgT": xgT, "xfT": xfT}


def kernel(**inputs):
    _patch_tile_drain()
    B = inputs["x_spatial"].shape[0]
    Bc = B // NCORES
    w = prep_weights(inputs)
    xb = prep_x(inputs)
    nc = build_program(Bc)
    in_maps = []
    for c in range(NCORES):
        m = dict(w)
        m["xspT"] = np.ascontiguousarray(xb["xspT"][:, :, c * Bc:(c + 1) * Bc])
        m["xgT"] = np.ascontiguousarray(xb["xgT"][:, c * Bc:(c + 1) * Bc])
        m["xfT"] = np.ascontiguousarray(xb["xfT"][:, c * Bc:(c + 1) * Bc])
        in_maps.append(m)
    res = run_bass_kernel_spmd(nc, in_maps, list(range(NCORES)))
    outs = [res.results[c]["outT"] for c in range(NCORES)]
    full = np.concatenate([o.T for o in outs], axis=0)
    return np.ascontiguousarray(full.astype(np.float32))
